# revision 14
# baseline (speedup 1.0000x reference)
"""Trainium2 Bass kernel for nn_LossFunction_12532714569881.

Computes, for x: [N=8192, 2, D=256] fp32, w, b scalars:
    P = x[:,0,:]; A = x[:,1,:]
    logits = (P @ A^T) / max(|p_i||a_j|, eps) * w + b        # [N, N]
    loss = -mean_i(log_softmax(logits)[i, i])
The additive b cancels in the row loss: loss_i = ln(sum_j e^{w cos_ij})
- w cos_ii, so no shift is needed anywhere (w cos in [-5, 5] for this
data, no overflow).

Strategy (8 NeuronCores, SPMD, single launch), V2 design:
  - Row-shard: core c owns rows r0=c*1024..+1024. Loads xp (its positive
    block) and xa (full anchors); its own anchor rows are sliced from xa.
  - All scale factors fold into the fp8 operands: positives are scaled
    by w*log2e/(2|p_i|)*kp, anchors by ka/|a_j| (kp=2, ka=8), so the
    matmul psum x satisfies exp(w cos) = 2^(x/8) directly.
  - fp8e4 DoubleRow matmuls (K=256 in one pass, ~1.7x bf16): weights are
    the transposed positives [ki, ko, m] with d=2ki+ko (built via two
    d-strided bf16 PE transposes + DVE copy); the moving operand is the
    anchor matrix pair-transposed IN A SINGLE DMA per 2048-column group:
    normalized fp8 anchors viewed as bf16 pairs, dma_start_transpose to
    [ki, t, j], re-viewed as fp8 [ki, ko(1B), j(2B)] - walrus accepts the
    byte-interleaved moving AP, so the PE never transposes anchors and
    nothing is copied out of PSUM.
  - exp + row-sum of each [128, 2048] psum tile runs on one of two
    engines to break the single-engine exp bottleneck (ACT is 1 elem/
    cycle): ACT units use Exp(scale=ln2/8) with fused accum; DVE units
    use a Schraudolph exponential - one tensor_scalar (x*16 + B -> int16
    = bf16 bits of 2^(x/8)) and one bf16 tensor_reduce at 2 elem/cycle.
    The Schraudolph magic B is calibrated so the loss bias under the
    randn input distribution is ~1e-7 (sensitivity ~7e-4 per 0.01).
  - Anchor norms: sum-of-squares on DVE/ACT (split per group), 1/|a| via
    Ln then Exp(-0.5x + ln ka) on ACT (one shared table set), normalize+
    fp8 cast on gpsimd (plain tensor_scalar; gpsimd accum paths do not
    work in this runtime).
  - The diagonal w*cos_ii is recomputed exactly in fp32 (DVE dots +
    norms), so fp8/Schraudolph noise only perturbs the log-sum-exp where
    it averages out. Expected rel err ~1e-4 (gate 2e-2).
  - Each core emits one partial scalar = sum of its 1024 row losses;
    the host sums and divides by N.
"""

import numpy as np

N = 8192
D = 256
NCORES = 8
RPC = N // NCORES          # 1024 rows per core
P = 128                    # partitions
NT_P = RPC // P            # 8 positive tiles / m-chunks
GCOLS = 2048               # columns per group
NGRP = N // GCOLS          # 4 column groups
TPG = GCOLS // P           # 16 anchor tiles per group
NB = 512                   # moving j-slice per DR matmul

LOG2E = 1.4426950408889634
KP = 2.0                   # positive fp8 scale headroom
KA = 8.0                   # anchor fp8 scale
SCHRA_C = 0.057101         # Schraudolph bias constant (calibrated)
A16 = 2.0 ** 23 / 8.0 / 65536.0          # = 16.0
B16 = (127.0 - SCHRA_C) * 128.0          # int16 magic
ACT_SCALE = float(np.log(2.0) / 8.0)     # exp(x*ln2/8) = 2^(x/8)

# exp unit assignment: units are (g, m); 'D' units run Schraudolph on DVE.
# Tuned for ACT/DVE balance (ACT ~2.0us/unit incl overhead, DVE ~3.4us).
DVE_UNITS = {(g, 3) for g in range(NGRP)} | {(2, 6), (3, 6)}
# sum-of-squares engine per anchor group: 'A' (ACT Square) or 'D' (DVE stt)
SUMSQ_ENG = ["A", "D", "D", "D"]

_BUILD_CACHE = {}
_ACT_TABLES_PATCHED = False
_LDW_OPT_PATCHED = False


def _patch_ldw_opt():
    """Enable walrus's redundant-LDWEIGHTS elision (hardcoded off in
    bass_utils); consecutive same-weight matmuls (our nn-runs of 4) then
    skip the PE array reload."""
    global _LDW_OPT_PATCHED
    if _LDW_OPT_PATCHED:
        return
    import concourse.bass_utils as bu

    orig_run = bu.run_command

    def patched(argv, **kwargs):
        argv = [a.replace("--enable-ldw-opt=false", "--enable-ldw-opt=true")
                if isinstance(a, str) else a for a in argv]
        return orig_run(argv, **kwargs)

    bu.run_command = patched
    _LDW_OPT_PATCHED = True


def _patch_act_tables():
    """Make Exp and Ln resolve to the one table set containing both, so a
    single ACT_TABLE_LOAD serves the whole kernel."""
    global _ACT_TABLES_PATCHED
    if _ACT_TABLES_PATCHED:
        return
    import concourse.bacc as bacc_mod
    import concourse.bass_interp as interp_mod
    import concourse.mybir as mybir
    from concourse import hw_specs

    AF = mybir.ActivationFunctionType
    orig = hw_specs.get_activation_tables

    def patched(module_arch):
        tabs = orig(module_arch)
        out = {}
        for name, funcs in tabs.items():
            f = set(funcs)
            if name != "natural_log_exp_and_others":
                f.discard(AF.Exp)
                f.discard(AF.Ln)
            out[name] = f
        return out

    bacc_mod.get_activation_tables = patched
    interp_mod.get_activation_tables = patched
    _ACT_TABLES_PATCHED = True


def _build(w: float, b: float):
    from contextlib import ExitStack

    import concourse.bass as bass  # noqa: F401
    import concourse.mybir as mybir
    import concourse.tile as tile
    from concourse import bacc

    _patch_act_tables()

    f32 = mybir.dt.float32
    f8 = mybir.dt.float8e4
    bf16 = mybir.dt.bfloat16
    i16 = mybir.dt.int16
    AF = mybir.ActivationFunctionType
    ALU = mybir.AluOpType
    AX = mybir.AxisListType
    MM = mybir.MatmulPerfMode

    nc = bacc.Bacc("TRN2", target_bir_lowering=False, debug=False)

    xp = nc.dram_tensor("xp", [RPC, D], f32, kind="ExternalInput").ap()
    xad = nc.dram_tensor("xad", [RPC, D], f32, kind="ExternalInput").ap()
    xa = nc.dram_tensor("xa", [N, D], f32, kind="ExternalInput").ap()
    out_partial = nc.dram_tensor("partial", [1, 1], f32, kind="ExternalOutput").ap()

    with tile.TileContext(nc) as tc:
        with ExitStack() as ctx:
            sing = ctx.enter_context(tc.tile_pool(name="sing", bufs=1))
            sq_pool = ctx.enter_context(tc.tile_pool(name="sqp", bufs=3))
            exp_pool = ctx.enter_context(tc.tile_pool(name="expp", bufs=2))
            ei_pool = ctx.enter_context(tc.tile_pool(name="eip", bufs=2))

            # ---- persistent SBUF tensors
            xa_raw = [sing.tile([P, TPG * D], f32, tag=f"xar{g}", name=f"xar{g}")
                      for g in range(NGRP)]
            a8 = [sing.tile([P, TPG * D], f8, tag=f"a8_{g}", name=f"a8_{g}")
                  for g in range(NGRP)]
            ant = [sing.tile([P, TPG, P], bf16, tag=f"ant{g}", name=f"ant{g}")
                   for g in range(NGRP)]
            ssq_a = [sing.tile([P, TPG], f32, tag=f"ssqa{g}", name=f"ssqa{g}")
                     for g in range(NGRP)]
            lns_a = [sing.tile([P, TPG], f32, tag=f"lnsa{g}", name=f"lnsa{g}")
                     for g in range(NGRP)]
            inv_a = [sing.tile([P, TPG], f32, tag=f"inva{g}", name=f"inva{g}")
                     for g in range(NGRP)]

            sb_xp = sing.tile([P, NT_P * D], f32, tag="xp")
            sb_xad = sing.tile([P, NT_P * D], f32, tag="xad")
            xpb = sing.tile([P, NT_P * D], bf16, tag="xpb")
            pnt3 = sing.tile([P, 2, RPC], f8, tag="pnt3")
            identb = sing.tile([P, P], bf16, tag="identb")

            ssq_ad = sing.tile([P, NT_P], f32, tag="ssqad")
            lns_ad = sing.tile([P, NT_P], f32, tag="lnsad")
            inv_ad = sing.tile([P, NT_P], f32, tag="invad")
            ssq_p = sing.tile([P, NT_P], f32, tag="ssqp")
            lns_p = sing.tile([P, NT_P], f32, tag="lnsp")
            inv_p = sing.tile([P, NT_P], f32, tag="invp")
            winvp = sing.tile([P, NT_P], f32, tag="winvp")
            pa = sing.tile([P, NT_P], f32, tag="pa")
            ssum = sing.tile([P, NT_P * NGRP], f32, tag="ssum")
            srow = sing.tile([P, NT_P], f32, tag="srow")
            lnS = sing.tile([P, NT_P], f32, tag="lnS")
            cosd = sing.tile([P, NT_P], f32, tag="cosd")
            rowloss = sing.tile([P, NT_P], f32, tag="rowloss")
            rsum = sing.tile([P, 1], f32, tag="rsum")
            ones = sing.tile([P, 1], f32, tag="ones")
            lnka_t = sing.tile([P, 1], f32, tag="lnka")
            sc_out = sing.tile([1, 1], f32, tag="sc_out")

            from concourse.masks import make_identity
            make_identity(nc, identb[:])
            nc.vector.memset(ones, 1.0)
            nc.vector.memset(lnka_t, float(np.log(KA)))

            # ---- loads: fine-grained chunks so compute starts early;
            # gpsimd's DMA queue stalls the machine - avoid it entirely.
            # Order: xp + xad first (gate P-prep/diag), then xa groups in
            # ascending order, each group split across both queues.
            # DMA issues share the engine instruction queues; keep the
            # early-critical loads (xp, xad, g0, g1) up front on sync/
            # scalar, and push g2/g3 loads to gpsimd (idle; slower pacing
            # is fine since that data is needed only ~60us in).
            xa_v = xa.rearrange("(g t p) d -> p g t d", p=P, t=TPG)

            def load_group(g, eng):
                for h in range(4):
                    eng.dma_start(
                        out=xa_raw[g].rearrange("p (t d) -> p t d", d=D)[:, h * 4:(h + 1) * 4, :],
                        in_=xa_v[:, g, h * 4:(h + 1) * 4, :],
                    )

            for h in range(2):
                nc.scalar.dma_start(
                    out=sb_xp.rearrange("p (t d) -> p t d", d=D)[:, h * 4:(h + 1) * 4, :],
                    in_=xp.rearrange("(t p) d -> p t d", p=P)[:, h * 4:(h + 1) * 4, :],
                )
            nc.sync.dma_start(
                out=sb_xad.rearrange("p (t d) -> p t d", d=D),
                in_=xad.rearrange("(t p) d -> p t d", p=P),
            )
            load_group(0, nc.sync)
            load_group(1, nc.scalar)
            load_group(2, nc.gpsimd)
            load_group(3, nc.gpsimd)

            def sumsq_dve(src, t, acc, col):
                scr = sq_pool.tile([P, D], f32, tag="sqscr", name="sqscr")
                nc.vector.scalar_tensor_tensor(
                    out=scr, in0=src[:, t * D:(t + 1) * D], scalar=1.0,
                    in1=src[:, t * D:(t + 1) * D],
                    op0=ALU.mult, op1=ALU.mult, accum_out=acc[:, col:col + 1],
                )

            def sumsq_act(src, t, acc, col):
                scr = sq_pool.tile([P, D], f32, tag="asqscr", name="asqscr")
                nc.scalar.activation(
                    scr, src[:, t * D:(t + 1) * D], AF.Square,
                    accum_out=acc[:, col:col + 1],
                )

            # ---- P-side prep: norms (fp32-exact), scaled bf16+fp8 casts
            for t in range(NT_P):
                sumsq_dve(sb_xp, t, ssq_p, t)
            nc.scalar.activation(lns_p, ssq_p, AF.Ln)
            nc.scalar.activation(inv_p, lns_p, AF.Exp, scale=-0.5)
            nc.vector.tensor_scalar_mul(winvp, inv_p, float(w) * LOG2E / 2.0 * KP)
            for t in range(NT_P):
                nc.vector.tensor_scalar_mul(
                    xpb[:, t * D:(t + 1) * D], sb_xp[:, t * D:(t + 1) * D],
                    winvp[:, t:t + 1])

            # transposed positives pnt3[ki, ko, m], d = 2*ki+ko, via PE
            with tc.tile_pool(name="psT", bufs=2, space="PSUM") as psT:
                for t in range(NT_P):
                    pst = psT.tile([P, 2, P], bf16, tag="pst", name="pst")
                    xv = xpb.rearrange("p (t d k) -> p t d k", t=NT_P, k=2)
                    for ko in range(2):
                        nc.tensor.transpose(pst[:, ko, :], xv[:, t, :, ko], identb)
                    nc.vector.tensor_copy(pnt3[:, :, t * P:(t + 1) * P], pst)

            # ---- diag dot (fp32 exact): pa_t = <p_i, a_i>
            # own anchor rows live in xa group gc, tiles toff..toff+7 ---
            # emitted late-ish; gated only on xa_raw[gc] load.

            # ---- per-group anchor pipeline + matmul/exp sweep.
            # Emission order pipelines prep one group ahead of the mm/exp
            # sweep so ACT/DVE queues interleave prep(g+1) with exp(g).
            def prep(g):
                # subgroups of 4 tiles pipeline through sumsq -> 1/|a| ->
                # normalize -> sub-transpose so the first matmuls of the
                # group start before the whole group is prepped.
                for s in range(TPG // 4):
                    t0 = s * 4
                    for t in range(t0, t0 + 4):
                        if SUMSQ_ENG[g] == "A":
                            sumsq_act(xa_raw[g], t, ssq_a[g], t)
                        else:
                            sumsq_dve(xa_raw[g], t, ssq_a[g], t)
                    nc.scalar.activation(lns_a[g][:, t0:t0 + 4],
                                         ssq_a[g][:, t0:t0 + 4], AF.Ln)
                    # 1/|a| * KA in one shot: exp(-0.5 ln ssq + ln KA)
                    nc.scalar.activation(inv_a[g][:, t0:t0 + 4],
                                         lns_a[g][:, t0:t0 + 4], AF.Exp,
                                         scale=-0.5, bias=lnka_t[:, 0:1])
                    # normalize + fp8 cast on DVE (2x_2p all-SBUF mode);
                    # gpsimd tensor ops are ~15x slower than spec here.
                    for t in range(t0, t0 + 4):
                        nc.vector.tensor_scalar_mul(
                            a8[g][:, t * D:(t + 1) * D],
                            xa_raw[g][:, t * D:(t + 1) * D],
                            inv_a[g][:, t:t + 1])
                    # pair-transpose the subgroup: bf16 view [j, (4 tiles)]
                    # -> ant[ki, t0:t0+4, j]
                    (nc.sync if g % 2 else nc.scalar).dma_start_transpose(
                        out=ant[g][:, t0:t0 + 4, :],
                        in_=a8[g].bitcast(bf16)[:, t0 * P:(t0 + 4) * P])

            def sweep(g, psM):
                rhs3 = ant[g].bitcast(f8).rearrange(
                    "p t (j k) -> p k (t j)", k=2)
                for m in range(NT_P):
                    ps = psM.tile([P, GCOLS], f32, tag="psmm", name="psmm")
                    for nn in range(GCOLS // NB):
                        nc.tensor.matmul(
                            ps[:, nn * NB:(nn + 1) * NB],
                            pnt3[:, :, m * P:(m + 1) * P],
                            rhs3[:, :, nn * NB:(nn + 1) * NB],
                            start=True, stop=True,
                            perf_mode=MM.DoubleRow,
                        )
                    ucol = m * NGRP + g
                    if (g, m) in DVE_UNITS:
                        ei = ei_pool.tile([P, GCOLS], i16, tag="ei", name="ei")
                        nc.vector.tensor_scalar(
                            out=ei, in0=ps, scalar1=A16, scalar2=B16,
                            op0=ALU.mult, op1=ALU.add)
                        scr2 = exp_pool.tile([P, GCOLS], bf16,
                                             tag="p2scr", name="p2scr")
                        nc.vector.tensor_scalar(
                            out=scr2, in0=ei.bitcast(bf16), scalar1=1.0,
                            scalar2=0.0, op0=ALU.mult, op1=ALU.add,
                            accum_out=ssum[:, ucol:ucol + 1])
                    else:
                        scr = exp_pool.tile([P, GCOLS], f8, tag="expscr",
                                            name="expscr")
                        nc.scalar.activation(
                            scr, ps, AF.Exp, scale=ACT_SCALE,
                            accum_out=ssum[:, ucol:ucol + 1])

            def diag_prep():
                for t in range(NT_P):
                    scr = sq_pool.tile([P, D], f32, tag="sqscr", name="sqscr")
                    nc.vector.scalar_tensor_tensor(
                        out=scr, in0=sb_xp[:, t * D:(t + 1) * D], scalar=1.0,
                        in1=sb_xad[:, t * D:(t + 1) * D],
                        op0=ALU.mult, op1=ALU.mult, accum_out=pa[:, t:t + 1],
                    )
                    sumsq_dve(sb_xad, t, ssq_ad, t)
                nc.scalar.activation(lns_ad, ssq_ad, AF.Ln)
                nc.scalar.activation(inv_ad, lns_ad, AF.Exp, scale=-0.5)

            with tc.tile_pool(name="psM", bufs=2, space="PSUM") as psM:
                prep(0)
                prep(1)
                diag_prep()
                sweep(0, psM)
                prep(2)
                sweep(1, psM)
                prep(3)
                sweep(2, psM)
                sweep(3, psM)

            # ---- tail -----------------------------------------------------
            nc.vector.tensor_reduce(
                srow, ssum.rearrange("p (m g) -> p m g", g=NGRP),
                axis=AX.X, op=ALU.add)
            nc.scalar.activation(lnS, srow, AF.Ln)
            # w*cos_ii = pa * inv_p * inv_ad * w
            nc.vector.tensor_mul(cosd, pa, inv_p)
            nc.vector.tensor_mul(cosd, cosd, inv_ad)
            nc.vector.tensor_scalar_mul(cosd, cosd, float(w))
            nc.vector.scalar_tensor_tensor(
                out=rowloss, in0=cosd, scalar=-1.0, in1=lnS,
                op0=ALU.mult, op1=ALU.add)
            nc.vector.reduce_sum(rsum, rowloss, axis=AX.X)
            with tc.tile_pool(name="psF", bufs=1, space="PSUM") as psF:
                pfin = psF.tile([1, 1], f32, tag="pfin")
                nc.tensor.matmul(pfin, rsum, ones, start=True, stop=True)
                nc.vector.tensor_copy(sc_out, pfin)
            nc.sync.dma_start(out=out_partial, in_=sc_out)

    nc.compile()
    return nc


def _get_nc(w: float, b: float):
    key = (float(w), float(b))
    if key not in _BUILD_CACHE:
        _BUILD_CACHE[key] = _build(float(w), float(b))
    return _BUILD_CACHE[key]


def kernel(x, w, b, epoch=None, **_unused):
    from concourse.bass_utils import run_bass_kernel_spmd

    x = np.asarray(x, dtype=np.float32)
    w_f = float(np.asarray(w))
    b_f = float(np.asarray(b))
    assert x.shape == (N, 2, D), x.shape

    nc = _get_nc(w_f, b_f)

    xa_full = np.ascontiguousarray(x[:, 1, :])
    in_maps = []
    for c in range(NCORES):
        r0 = c * RPC
        in_maps.append({
            "xp": np.ascontiguousarray(x[r0:r0 + RPC, 0, :]),
            "xad": np.ascontiguousarray(x[r0:r0 + RPC, 1, :]),
            "xa": xa_full,
        })

    res = run_bass_kernel_spmd(nc, in_maps, list(range(NCORES)))
    total = 0.0
    for c in range(NCORES):
        total += float(res.results[c]["partial"][0, 0])
    loss = total / N
    return np.float32(loss)


# revision 16
# speedup vs baseline: 1.0384x; 1.0384x over previous
"""Trainium2 Bass kernel for nn_LossFunction_12532714569881.

Computes, for x: [N=8192, 2, D=256] fp32, w, b scalars:
    P = x[:,0,:]; A = x[:,1,:]
    logits = (P @ A^T) / max(|p_i||a_j|, eps) * w + b        # [N, N]
    loss = -mean_i(log_softmax(logits)[i, i])
The additive b cancels in the row loss: loss_i = ln(sum_j e^{w cos_ij})
- w cos_ii, so no shift is needed anywhere (w cos in [-5, 5] for this
data, no overflow).

Strategy (8 NeuronCores, SPMD, single launch), V2 design:
  - Row-shard: core c owns rows r0=c*1024..+1024. Loads xp (its positive
    block) and xa (full anchors); its own anchor rows are sliced from xa.
  - All scale factors fold into the fp8 operands: positives are scaled
    by w*log2e/(2|p_i|)*kp, anchors by ka/|a_j| (kp=2, ka=8), so the
    matmul psum x satisfies exp(w cos) = 2^(x/8) directly.
  - fp8e4 DoubleRow matmuls (K=256 in one pass, ~1.7x bf16): weights are
    the transposed positives [ki, ko, m] with d=2ki+ko (built via two
    d-strided bf16 PE transposes + DVE copy); the moving operand is the
    anchor matrix pair-transposed IN A SINGLE DMA per 2048-column group:
    normalized fp8 anchors viewed as bf16 pairs, dma_start_transpose to
    [ki, t, j], re-viewed as fp8 [ki, ko(1B), j(2B)] - walrus accepts the
    byte-interleaved moving AP, so the PE never transposes anchors and
    nothing is copied out of PSUM.
  - exp + row-sum of each [128, 2048] psum tile runs on one of two
    engines to break the single-engine exp bottleneck (ACT is 1 elem/
    cycle): ACT units use Exp(scale=ln2/8) with fused accum; DVE units
    use a Schraudolph exponential - one tensor_scalar (x*16 + B -> int16
    = bf16 bits of 2^(x/8)) and one bf16 tensor_reduce at 2 elem/cycle.
    The Schraudolph magic B is calibrated so the loss bias under the
    randn input distribution is ~1e-7 (sensitivity ~7e-4 per 0.01).
  - Anchor norms: sum-of-squares on DVE/ACT (split per group), 1/|a| via
    Ln then Exp(-0.5x + ln ka) on ACT (one shared table set), normalize+
    fp8 cast on gpsimd (plain tensor_scalar; gpsimd accum paths do not
    work in this runtime).
  - The diagonal w*cos_ii is recomputed exactly in fp32 (DVE dots +
    norms), so fp8/Schraudolph noise only perturbs the log-sum-exp where
    it averages out. Expected rel err ~1e-4 (gate 2e-2).
  - Each core emits one partial scalar = sum of its 1024 row losses;
    the host sums and divides by N.
"""

import numpy as np

N = 8192
D = 256
NCORES = 8
RPC = N // NCORES          # 1024 rows per core
P = 128                    # partitions
NT_P = RPC // P            # 8 positive tiles / m-chunks
GCOLS = 2048               # columns per group
NGRP = N // GCOLS          # 4 column groups
TPG = GCOLS // P           # 16 anchor tiles per group
NB = 512                   # moving j-slice per DR matmul

LOG2E = 1.4426950408889634
KP = 2.0                   # positive fp8 scale headroom
KA = 8.0                   # anchor fp8 scale
SCHRA_C = 0.057101         # Schraudolph bias constant (calibrated)
A16 = 2.0 ** 23 / 8.0 / 65536.0          # = 16.0
B16 = (127.0 - SCHRA_C) * 128.0          # int16 magic
ACT_SCALE = float(np.log(2.0) / 8.0)     # exp(x*ln2/8) = 2^(x/8)

# exp unit assignment: units are (g, m); 'D' units run Schraudolph on DVE.
# Tuned for ACT/DVE balance (ACT ~2.0us/unit incl overhead, DVE ~3.4us).
DVE_UNITS = {(g, 3) for g in range(NGRP)} | {(2, 6), (3, 6)}
# sum-of-squares engine per anchor group: 'A' (ACT Square) or 'D' (DVE stt)
SUMSQ_ENG = ["A", "D", "D", "D"]

_BUILD_CACHE = {}
_ACT_TABLES_PATCHED = False
_LDW_OPT_PATCHED = False


def _patch_ldw_opt():
    """Enable walrus's redundant-LDWEIGHTS elision (hardcoded off in
    bass_utils); consecutive same-weight matmuls (our nn-runs of 4) then
    skip the PE array reload."""
    global _LDW_OPT_PATCHED
    if _LDW_OPT_PATCHED:
        return
    import concourse.bass_utils as bu

    orig_run = bu.run_command

    def patched(argv, **kwargs):
        argv = [a.replace("--enable-ldw-opt=false", "--enable-ldw-opt=true")
                if isinstance(a, str) else a for a in argv]
        return orig_run(argv, **kwargs)

    bu.run_command = patched
    _LDW_OPT_PATCHED = True


def _patch_act_tables():
    """Make Exp and Ln resolve to the one table set containing both, so a
    single ACT_TABLE_LOAD serves the whole kernel."""
    global _ACT_TABLES_PATCHED
    if _ACT_TABLES_PATCHED:
        return
    import concourse.bacc as bacc_mod
    import concourse.bass_interp as interp_mod
    import concourse.mybir as mybir
    from concourse import hw_specs

    AF = mybir.ActivationFunctionType
    orig = hw_specs.get_activation_tables

    def patched(module_arch):
        tabs = orig(module_arch)
        out = {}
        for name, funcs in tabs.items():
            f = set(funcs)
            if name != "natural_log_exp_and_others":
                f.discard(AF.Exp)
                f.discard(AF.Ln)
            out[name] = f
        return out

    bacc_mod.get_activation_tables = patched
    interp_mod.get_activation_tables = patched
    _ACT_TABLES_PATCHED = True


def _build(w: float, b: float):
    from contextlib import ExitStack

    import concourse.bass as bass  # noqa: F401
    import concourse.mybir as mybir
    import concourse.tile as tile
    from concourse import bacc

    _patch_act_tables()

    f32 = mybir.dt.float32
    f8 = mybir.dt.float8e4
    bf16 = mybir.dt.bfloat16
    i16 = mybir.dt.int16
    AF = mybir.ActivationFunctionType
    ALU = mybir.AluOpType
    AX = mybir.AxisListType
    MM = mybir.MatmulPerfMode

    nc = bacc.Bacc("TRN2", target_bir_lowering=False, debug=False)

    xp = nc.dram_tensor("xp", [RPC, D], f32, kind="ExternalInput").ap()
    xad = nc.dram_tensor("xad", [RPC, D], f32, kind="ExternalInput").ap()
    xa = nc.dram_tensor("xa", [N, D], f32, kind="ExternalInput").ap()
    out_partial = nc.dram_tensor("partial", [1, 1], f32, kind="ExternalOutput").ap()

    with tile.TileContext(nc) as tc:
        with ExitStack() as ctx:
            sing = ctx.enter_context(tc.tile_pool(name="sing", bufs=1))
            sq_pool = ctx.enter_context(tc.tile_pool(name="sqp", bufs=3))
            exp_pool = ctx.enter_context(tc.tile_pool(name="expp", bufs=2))
            ei_pool = ctx.enter_context(tc.tile_pool(name="eip", bufs=2))

            # ---- persistent SBUF tensors
            xa_raw = [sing.tile([P, TPG * D], f32, tag=f"xar{g}", name=f"xar{g}")
                      for g in range(NGRP)]
            a8 = [sing.tile([P, TPG * D], f8, tag=f"a8_{g}", name=f"a8_{g}")
                  for g in range(NGRP)]
            ant = [sing.tile([P, TPG, P], bf16, tag=f"ant{g}", name=f"ant{g}")
                   for g in range(NGRP)]
            ssq_a = [sing.tile([P, TPG], f32, tag=f"ssqa{g}", name=f"ssqa{g}")
                     for g in range(NGRP)]
            lns_a = [sing.tile([P, TPG], f32, tag=f"lnsa{g}", name=f"lnsa{g}")
                     for g in range(NGRP)]
            inv_a = [sing.tile([P, TPG], f32, tag=f"inva{g}", name=f"inva{g}")
                     for g in range(NGRP)]

            sb_xp = sing.tile([P, NT_P * D], f32, tag="xp")
            sb_xad = sing.tile([P, NT_P * D], f32, tag="xad")
            xpb = sing.tile([P, NT_P * D], bf16, tag="xpb")
            pnt3 = sing.tile([P, 2, RPC], f8, tag="pnt3")
            identb = sing.tile([P, P], bf16, tag="identb")

            ssq_ad = sing.tile([P, NT_P], f32, tag="ssqad")
            lns_ad = sing.tile([P, NT_P], f32, tag="lnsad")
            inv_ad = sing.tile([P, NT_P], f32, tag="invad")
            ssq_p = sing.tile([P, NT_P], f32, tag="ssqp")
            lns_p = sing.tile([P, NT_P], f32, tag="lnsp")
            inv_p = sing.tile([P, NT_P], f32, tag="invp")
            winvp = sing.tile([P, NT_P], f32, tag="winvp")
            pa = sing.tile([P, NT_P], f32, tag="pa")
            ssum = sing.tile([P, NT_P * NGRP], f32, tag="ssum")
            srow = sing.tile([P, NT_P], f32, tag="srow")
            lnS = sing.tile([P, NT_P], f32, tag="lnS")
            cosd = sing.tile([P, NT_P], f32, tag="cosd")
            rowloss = sing.tile([P, NT_P], f32, tag="rowloss")
            rsum = sing.tile([P, 1], f32, tag="rsum")
            ones = sing.tile([P, 1], f32, tag="ones")
            lnka_t = sing.tile([P, 1], f32, tag="lnka")
            sc_out = sing.tile([1, 1], f32, tag="sc_out")

            from concourse.masks import make_identity
            make_identity(nc, identb[:])
            nc.vector.memset(ones, 1.0)
            nc.vector.memset(lnka_t, float(np.log(KA)))

            # ---- loads: fine-grained chunks so compute starts early;
            # gpsimd's DMA queue stalls the machine - avoid it entirely.
            # Order: xp + xad first (gate P-prep/diag), then xa groups in
            # ascending order, each group split across both queues.
            # DMA issues share the engine instruction queues; keep the
            # early-critical loads (xp, xad, g0, g1) up front on sync/
            # scalar, and push g2/g3 loads to gpsimd (idle; slower pacing
            # is fine since that data is needed only ~60us in).
            xa_v = xa.rearrange("(g t p) d -> p g t d", p=P, t=TPG)

            def load_group(g, eng):
                for h in range(4):
                    eng.dma_start(
                        out=xa_raw[g].rearrange("p (t d) -> p t d", d=D)[:, h * 4:(h + 1) * 4, :],
                        in_=xa_v[:, g, h * 4:(h + 1) * 4, :],
                    )

            # Per-queue DMA bandwidth is ~150GB/s, so the early-critical
            # data (xp, g0) is split across the scalar+sync queues while
            # later groups ride the vector/tensor/gpsimd queues (idle at
            # t=0; their issue cost there is negligible).
            for h in range(2):
                nc.scalar.dma_start(
                    out=sb_xp.rearrange("p (t d) -> p t d", d=D)[:, h * 4:(h + 1) * 4, :],
                    in_=xp.rearrange("(t p) d -> p t d", p=P)[:, h * 4:(h + 1) * 4, :],
                )
            for h in range(4):
                (nc.sync if h < 2 else nc.scalar).dma_start(
                    out=xa_raw[0].rearrange("p (t d) -> p t d", d=D)[:, h * 4:(h + 1) * 4, :],
                    in_=xa_v[:, 0, h * 4:(h + 1) * 4, :],
                )
            nc.gpsimd.dma_start(
                out=sb_xad.rearrange("p (t d) -> p t d", d=D),
                in_=xad.rearrange("(t p) d -> p t d", p=P),
            )
            load_group(1, nc.sync)
            load_group(2, nc.gpsimd)
            load_group(3, nc.gpsimd)

            def sumsq_dve(src, t, acc, col):
                scr = sq_pool.tile([P, D], f32, tag="sqscr", name="sqscr")
                nc.vector.scalar_tensor_tensor(
                    out=scr, in0=src[:, t * D:(t + 1) * D], scalar=1.0,
                    in1=src[:, t * D:(t + 1) * D],
                    op0=ALU.mult, op1=ALU.mult, accum_out=acc[:, col:col + 1],
                )

            def sumsq_act(src, t, acc, col):
                scr = sq_pool.tile([P, D], f32, tag="asqscr", name="asqscr")
                nc.scalar.activation(
                    scr, src[:, t * D:(t + 1) * D], AF.Square,
                    accum_out=acc[:, col:col + 1],
                )

            # ---- P-side prep: norms (fp32-exact), scaled bf16+fp8 casts
            for t in range(NT_P):
                sumsq_dve(sb_xp, t, ssq_p, t)
            nc.scalar.activation(lns_p, ssq_p, AF.Ln)
            nc.scalar.activation(inv_p, lns_p, AF.Exp, scale=-0.5)
            nc.vector.tensor_scalar_mul(winvp, inv_p, float(w) * LOG2E / 2.0 * KP)
            for t in range(NT_P):
                nc.vector.tensor_scalar_mul(
                    xpb[:, t * D:(t + 1) * D], sb_xp[:, t * D:(t + 1) * D],
                    winvp[:, t:t + 1])

            # transposed positives pnt3[ki, ko, m], d = 2*ki+ko, via PE
            with tc.tile_pool(name="psT", bufs=2, space="PSUM") as psT:
                for t in range(NT_P):
                    pst = psT.tile([P, 2, P], bf16, tag="pst", name="pst")
                    xv = xpb.rearrange("p (t d k) -> p t d k", t=NT_P, k=2)
                    for ko in range(2):
                        nc.tensor.transpose(pst[:, ko, :], xv[:, t, :, ko], identb)
                    nc.vector.tensor_copy(pnt3[:, :, t * P:(t + 1) * P], pst)

            # ---- diag dot (fp32 exact): pa_t = <p_i, a_i>
            # own anchor rows live in xa group gc, tiles toff..toff+7 ---
            # emitted late-ish; gated only on xa_raw[gc] load.

            # ---- per-group anchor pipeline + matmul/exp sweep.
            # Emission order pipelines prep one group ahead of the mm/exp
            # sweep so ACT/DVE queues interleave prep(g+1) with exp(g).
            def prep(g):
                # subgroups of 4 tiles pipeline through sumsq -> 1/|a| ->
                # normalize -> sub-transpose so the first matmuls of the
                # group start before the whole group is prepped.
                for s in range(TPG // 4):
                    t0 = s * 4
                    for t in range(t0, t0 + 4):
                        if SUMSQ_ENG[g] == "A":
                            sumsq_act(xa_raw[g], t, ssq_a[g], t)
                        else:
                            sumsq_dve(xa_raw[g], t, ssq_a[g], t)
                    nc.scalar.activation(lns_a[g][:, t0:t0 + 4],
                                         ssq_a[g][:, t0:t0 + 4], AF.Ln)
                    # 1/|a| * KA in one shot: exp(-0.5 ln ssq + ln KA)
                    nc.scalar.activation(inv_a[g][:, t0:t0 + 4],
                                         lns_a[g][:, t0:t0 + 4], AF.Exp,
                                         scale=-0.5, bias=lnka_t[:, 0:1])
                    # normalize + fp8 cast on DVE (2x_2p all-SBUF mode);
                    # gpsimd tensor ops are ~15x slower than spec here.
                    for t in range(t0, t0 + 4):
                        nc.vector.tensor_scalar_mul(
                            a8[g][:, t * D:(t + 1) * D],
                            xa_raw[g][:, t * D:(t + 1) * D],
                            inv_a[g][:, t:t + 1])
                    # pair-transpose the subgroup: bf16 view [j, (4 tiles)]
                    # -> ant[ki, t0:t0+4, j]
                    (nc.sync if g % 2 else nc.scalar).dma_start_transpose(
                        out=ant[g][:, t0:t0 + 4, :],
                        in_=a8[g].bitcast(bf16)[:, t0 * P:(t0 + 4) * P])

            def sweep(g, psM):
                rhs3 = ant[g].bitcast(f8).rearrange(
                    "p t (j k) -> p k (t j)", k=2)
                for m in range(NT_P):
                    ps = psM.tile([P, GCOLS], f32, tag="psmm", name="psmm")
                    for nn in range(GCOLS // NB):
                        nc.tensor.matmul(
                            ps[:, nn * NB:(nn + 1) * NB],
                            pnt3[:, :, m * P:(m + 1) * P],
                            rhs3[:, :, nn * NB:(nn + 1) * NB],
                            start=True, stop=True,
                            perf_mode=MM.DoubleRow,
                        )
                    ucol = m * NGRP + g
                    if (g, m) in DVE_UNITS:
                        ei = ei_pool.tile([P, GCOLS], i16, tag="ei", name="ei")
                        nc.vector.tensor_scalar(
                            out=ei, in0=ps, scalar1=A16, scalar2=B16,
                            op0=ALU.mult, op1=ALU.add)
                        scr2 = exp_pool.tile([P, GCOLS], bf16,
                                             tag="p2scr", name="p2scr")
                        nc.vector.tensor_scalar(
                            out=scr2, in0=ei.bitcast(bf16), scalar1=1.0,
                            scalar2=0.0, op0=ALU.mult, op1=ALU.add,
                            accum_out=ssum[:, ucol:ucol + 1])
                    else:
                        scr = exp_pool.tile([P, GCOLS], f8, tag="expscr",
                                            name="expscr")
                        nc.scalar.activation(
                            scr, ps, AF.Exp, scale=ACT_SCALE,
                            accum_out=ssum[:, ucol:ucol + 1])

            def diag_prep():
                for t in range(NT_P):
                    scr = sq_pool.tile([P, D], f32, tag="sqscr", name="sqscr")
                    nc.vector.scalar_tensor_tensor(
                        out=scr, in0=sb_xp[:, t * D:(t + 1) * D], scalar=1.0,
                        in1=sb_xad[:, t * D:(t + 1) * D],
                        op0=ALU.mult, op1=ALU.mult, accum_out=pa[:, t:t + 1],
                    )
                    sumsq_dve(sb_xad, t, ssq_ad, t)
                nc.scalar.activation(lns_ad, ssq_ad, AF.Ln)
                nc.scalar.activation(inv_ad, lns_ad, AF.Exp, scale=-0.5)

            with tc.tile_pool(name="psM", bufs=2, space="PSUM") as psM:
                prep(0)
                prep(1)
                diag_prep()
                sweep(0, psM)
                prep(2)
                sweep(1, psM)
                prep(3)
                sweep(2, psM)
                sweep(3, psM)

            # ---- tail -----------------------------------------------------
            nc.vector.tensor_reduce(
                srow, ssum.rearrange("p (m g) -> p m g", g=NGRP),
                axis=AX.X, op=ALU.add)
            nc.scalar.activation(lnS, srow, AF.Ln)
            # w*cos_ii = pa * inv_p * inv_ad * w
            nc.vector.tensor_mul(cosd, pa, inv_p)
            nc.vector.tensor_mul(cosd, cosd, inv_ad)
            nc.vector.tensor_scalar_mul(cosd, cosd, float(w))
            nc.vector.scalar_tensor_tensor(
                out=rowloss, in0=cosd, scalar=-1.0, in1=lnS,
                op0=ALU.mult, op1=ALU.add)
            nc.vector.reduce_sum(rsum, rowloss, axis=AX.X)
            with tc.tile_pool(name="psF", bufs=1, space="PSUM") as psF:
                pfin = psF.tile([1, 1], f32, tag="pfin")
                nc.tensor.matmul(pfin, rsum, ones, start=True, stop=True)
                nc.vector.tensor_copy(sc_out, pfin)
            nc.sync.dma_start(out=out_partial, in_=sc_out)

    nc.compile()
    return nc


def _get_nc(w: float, b: float):
    key = (float(w), float(b))
    if key not in _BUILD_CACHE:
        _BUILD_CACHE[key] = _build(float(w), float(b))
    return _BUILD_CACHE[key]


def kernel(x, w, b, epoch=None, **_unused):
    from concourse.bass_utils import run_bass_kernel_spmd

    x = np.asarray(x, dtype=np.float32)
    w_f = float(np.asarray(w))
    b_f = float(np.asarray(b))
    assert x.shape == (N, 2, D), x.shape

    nc = _get_nc(w_f, b_f)

    xa_full = np.ascontiguousarray(x[:, 1, :])
    in_maps = []
    for c in range(NCORES):
        r0 = c * RPC
        in_maps.append({
            "xp": np.ascontiguousarray(x[r0:r0 + RPC, 0, :]),
            "xad": np.ascontiguousarray(x[r0:r0 + RPC, 1, :]),
            "xa": xa_full,
        })

    res = run_bass_kernel_spmd(nc, in_maps, list(range(NCORES)))
    total = 0.0
    for c in range(NCORES):
        total += float(res.results[c]["partial"][0, 0])
    loss = total / N
    return np.float32(loss)


# revision 18
# speedup vs baseline: 1.0965x; 1.0559x over previous
"""Trainium2 Bass kernel for nn_LossFunction_12532714569881.

Computes, for x: [N=8192, 2, D=256] fp32, w, b scalars:
    P = x[:,0,:]; A = x[:,1,:]
    logits = (P @ A^T) / max(|p_i||a_j|, eps) * w + b        # [N, N]
    loss = -mean_i(log_softmax(logits)[i, i])
The additive b cancels in the row loss: loss_i = ln(sum_j e^{w cos_ij})
- w cos_ii, so no shift is needed anywhere (w cos in [-5, 5] for this
data, no overflow).

Strategy (8 NeuronCores, SPMD, single launch), V2 design:
  - Row-shard: core c owns rows r0=c*1024..+1024. Loads xp (its positive
    block) and xa (full anchors); its own anchor rows are sliced from xa.
  - All scale factors fold into the fp8 operands: positives are scaled
    by w*log2e/(2|p_i|)*kp, anchors by ka/|a_j| (kp=2, ka=8), so the
    matmul psum x satisfies exp(w cos) = 2^(x/8) directly.
  - fp8e4 DoubleRow matmuls (K=256 in one pass, ~1.7x bf16): weights are
    the transposed positives [ki, ko, m] with d=2ki+ko (built via two
    d-strided bf16 PE transposes + DVE copy); the moving operand is the
    anchor matrix pair-transposed IN A SINGLE DMA per 2048-column group:
    normalized fp8 anchors viewed as bf16 pairs, dma_start_transpose to
    [ki, t, j], re-viewed as fp8 [ki, ko(1B), j(2B)] - walrus accepts the
    byte-interleaved moving AP, so the PE never transposes anchors and
    nothing is copied out of PSUM.
  - exp + row-sum of each [128, 2048] psum tile runs on one of two
    engines to break the single-engine exp bottleneck (ACT is 1 elem/
    cycle): ACT units use Exp(scale=ln2/8) with fused accum; DVE units
    use a Schraudolph exponential - one tensor_scalar (x*16 + B -> int16
    = bf16 bits of 2^(x/8)) and one bf16 tensor_reduce at 2 elem/cycle.
    The Schraudolph magic B is calibrated so the loss bias under the
    randn input distribution is ~1e-7 (sensitivity ~7e-4 per 0.01).
  - Anchor norms: sum-of-squares on DVE/ACT (split per group), 1/|a| via
    Ln then Exp(-0.5x + ln ka) on ACT (one shared table set), normalize+
    fp8 cast on gpsimd (plain tensor_scalar; gpsimd accum paths do not
    work in this runtime).
  - The diagonal w*cos_ii is recomputed exactly in fp32 (DVE dots +
    norms), so fp8/Schraudolph noise only perturbs the log-sum-exp where
    it averages out. Expected rel err ~1e-4 (gate 2e-2).
  - Each core emits one partial scalar = sum of its 1024 row losses;
    the host sums and divides by N.
"""

import numpy as np

N = 8192
D = 256
NCORES = 8
RPC = N // NCORES          # 1024 rows per core
P = 128                    # partitions
NT_P = RPC // P            # 8 positive tiles / m-chunks
GCOLS = 2048               # columns per group
NGRP = N // GCOLS          # 4 column groups
TPG = GCOLS // P           # 16 anchor tiles per group
NB = 512                   # moving j-slice per DR matmul

LOG2E = 1.4426950408889634
KP = 2.0                   # positive fp8 scale headroom
KA = 8.0                   # anchor fp8 scale
SCHRA_C = 0.057101         # Schraudolph bias constant (calibrated)
A16 = 2.0 ** 23 / 8.0 / 65536.0          # = 16.0
B16 = (127.0 - SCHRA_C) * 128.0          # int16 magic
ACT_SCALE = float(np.log(2.0) / 8.0)     # exp(x*ln2/8) = 2^(x/8)

# exp unit assignment: units are (g, m); 'D' units run Schraudolph on DVE.
# Tuned for ACT/DVE balance (ACT ~2.0us/unit incl overhead, DVE ~3.4us).
DVE_UNITS = {(g, 3) for g in range(NGRP)} | {(2, 6), (3, 6)}
# sum-of-squares engine per anchor group: 'A' (ACT Square) or 'D' (DVE stt)
SUMSQ_ENG = ["A", "D", "D", "D"]

_BUILD_CACHE = {}
_ACT_TABLES_PATCHED = False
_LDW_OPT_PATCHED = False


def _patch_ldw_opt():
    """Enable walrus's redundant-LDWEIGHTS elision (hardcoded off in
    bass_utils); consecutive same-weight matmuls (our nn-runs of 4) then
    skip the PE array reload."""
    global _LDW_OPT_PATCHED
    if _LDW_OPT_PATCHED:
        return
    import concourse.bass_utils as bu

    orig_run = bu.run_command

    def patched(argv, **kwargs):
        argv = [a.replace("--enable-ldw-opt=false", "--enable-ldw-opt=true")
                if isinstance(a, str) else a for a in argv]
        return orig_run(argv, **kwargs)

    bu.run_command = patched
    _LDW_OPT_PATCHED = True


def _patch_act_tables():
    """Make Exp and Ln resolve to the one table set containing both, so a
    single ACT_TABLE_LOAD serves the whole kernel."""
    global _ACT_TABLES_PATCHED
    if _ACT_TABLES_PATCHED:
        return
    import concourse.bacc as bacc_mod
    import concourse.bass_interp as interp_mod
    import concourse.mybir as mybir
    from concourse import hw_specs

    AF = mybir.ActivationFunctionType
    orig = hw_specs.get_activation_tables

    def patched(module_arch):
        tabs = orig(module_arch)
        out = {}
        for name, funcs in tabs.items():
            f = set(funcs)
            if name != "natural_log_exp_and_others":
                f.discard(AF.Exp)
                f.discard(AF.Ln)
            out[name] = f
        return out

    bacc_mod.get_activation_tables = patched
    interp_mod.get_activation_tables = patched
    _ACT_TABLES_PATCHED = True


def _build(w: float, b: float):
    from contextlib import ExitStack

    import concourse.bass as bass  # noqa: F401
    import concourse.mybir as mybir
    import concourse.tile as tile
    from concourse import bacc

    _patch_act_tables()

    f32 = mybir.dt.float32
    f8 = mybir.dt.float8e4
    bf16 = mybir.dt.bfloat16
    i16 = mybir.dt.int16
    AF = mybir.ActivationFunctionType
    ALU = mybir.AluOpType
    AX = mybir.AxisListType
    MM = mybir.MatmulPerfMode

    nc = bacc.Bacc("TRN2", target_bir_lowering=False, debug=False)

    xp = nc.dram_tensor("xp", [RPC, D], f32, kind="ExternalInput").ap()
    xa = nc.dram_tensor("xa", [N, D], f32, kind="ExternalInput").ap()
    out_partial = nc.dram_tensor("partial", [1, 1], f32, kind="ExternalOutput").ap()

    with tile.TileContext(nc) as tc:
        with ExitStack() as ctx:
            sing = ctx.enter_context(tc.tile_pool(name="sing", bufs=1))
            sq_pool = ctx.enter_context(tc.tile_pool(name="sqp", bufs=3))
            exp_pool = ctx.enter_context(tc.tile_pool(name="expp", bufs=2))
            ei_pool = ctx.enter_context(tc.tile_pool(name="eip", bufs=2))

            # ---- persistent SBUF tensors
            xa_raw = [sing.tile([P, TPG * D], f32, tag=f"xar{g}", name=f"xar{g}")
                      for g in range(NGRP)]
            a8 = [sing.tile([P, TPG * D], f8, tag=f"a8_{g}", name=f"a8_{g}")
                  for g in range(NGRP)]
            ant = [sing.tile([P, TPG, P], bf16, tag=f"ant{g}", name=f"ant{g}")
                   for g in range(NGRP)]
            ssq_a = [sing.tile([P, TPG], f32, tag=f"ssqa{g}", name=f"ssqa{g}")
                     for g in range(NGRP)]
            lns_a = [sing.tile([P, TPG], f32, tag=f"lnsa{g}", name=f"lnsa{g}")
                     for g in range(NGRP)]
            inv_a = [sing.tile([P, TPG], f32, tag=f"inva{g}", name=f"inva{g}")
                     for g in range(NGRP)]

            sb_xp = sing.tile([P, NT_P * D], f32, tag="xp")
            xpb = sing.tile([P, NT_P * D], bf16, tag="xpb")
            pnt3 = sing.tile([P, 2, RPC], f8, tag="pnt3")
            identb = sing.tile([P, P], bf16, tag="identb")

            ssq_p = sing.tile([P, NT_P], f32, tag="ssqp")
            lns_p = sing.tile([P, NT_P], f32, tag="lnsp")
            inv_p = sing.tile([P, NT_P], f32, tag="invp")
            winvp = sing.tile([P, NT_P], f32, tag="winvp")
            pa = sing.tile([P, NT_P], f32, tag="pa")
            ssum = sing.tile([P, NT_P * NGRP], f32, tag="ssum")
            srow = sing.tile([P, NT_P], f32, tag="srow")
            lnS = sing.tile([P, NT_P], f32, tag="lnS")
            cosd = sing.tile([P, NT_P], f32, tag="cosd")
            rowloss = sing.tile([P, NT_P], f32, tag="rowloss")
            rsum = sing.tile([P, 1], f32, tag="rsum")
            ones = sing.tile([P, 1], f32, tag="ones")
            lnka_t = sing.tile([P, 1], f32, tag="lnka")
            sc_out = sing.tile([1, 1], f32, tag="sc_out")

            from concourse.masks import make_identity
            make_identity(nc, identb[:])
            nc.vector.memset(ones, 1.0)
            nc.vector.memset(lnka_t, float(np.log(KA)))

            # ---- loads: fine-grained chunks so compute starts early;
            # gpsimd's DMA queue stalls the machine - avoid it entirely.
            # Order: xp + xad first (gate P-prep/diag), then xa groups in
            # ascending order, each group split across both queues.
            # DMA issues share the engine instruction queues; keep the
            # early-critical loads (xp, xad, g0, g1) up front on sync/
            # scalar, and push g2/g3 loads to gpsimd (idle; slower pacing
            # is fine since that data is needed only ~60us in).
            xa_v = xa.rearrange("(g t p) d -> p g t d", p=P, t=TPG)

            def load_group(g, eng):
                for h in range(4):
                    eng.dma_start(
                        out=xa_raw[g].rearrange("p (t d) -> p t d", d=D)[:, h * 4:(h + 1) * 4, :],
                        in_=xa_v[:, g, h * 4:(h + 1) * 4, :],
                    )

            # Per-queue DMA bandwidth is ~150GB/s, so the early-critical
            # data (xp, g0) is split across the scalar+sync queues while
            # later groups ride the vector/tensor/gpsimd queues (idle at
            # t=0; their issue cost there is negligible).
            for h in range(2):
                nc.scalar.dma_start(
                    out=sb_xp.rearrange("p (t d) -> p t d", d=D)[:, h * 4:(h + 1) * 4, :],
                    in_=xp.rearrange("(t p) d -> p t d", p=P)[:, h * 4:(h + 1) * 4, :],
                )
            for h in range(4):
                (nc.sync if h < 2 else nc.scalar).dma_start(
                    out=xa_raw[0].rearrange("p (t d) -> p t d", d=D)[:, h * 4:(h + 1) * 4, :],
                    in_=xa_v[:, 0, h * 4:(h + 1) * 4, :],
                )
            load_group(1, nc.sync)
            load_group(2, nc.gpsimd)
            load_group(3, nc.gpsimd)

            def sumsq_dve(src, t, acc, col):
                scr = sq_pool.tile([P, D], f32, tag="sqscr", name="sqscr")
                nc.vector.scalar_tensor_tensor(
                    out=scr, in0=src[:, t * D:(t + 1) * D], scalar=1.0,
                    in1=src[:, t * D:(t + 1) * D],
                    op0=ALU.mult, op1=ALU.mult, accum_out=acc[:, col:col + 1],
                )

            def sumsq_act(src, t, acc, col):
                scr = sq_pool.tile([P, D], f32, tag="asqscr", name="asqscr")
                nc.scalar.activation(
                    scr, src[:, t * D:(t + 1) * D], AF.Square,
                    accum_out=acc[:, col:col + 1],
                )

            # ---- P-side prep: norms (fp32-exact), scaled bf16+fp8 casts
            for t in range(NT_P):
                sumsq_dve(sb_xp, t, ssq_p, t)
            nc.scalar.activation(lns_p, ssq_p, AF.Ln)
            nc.scalar.activation(inv_p, lns_p, AF.Exp, scale=-0.5)
            nc.vector.tensor_scalar_mul(winvp, inv_p, float(w) * LOG2E / 2.0 * KP)
            for t in range(NT_P):
                nc.vector.tensor_scalar_mul(
                    xpb[:, t * D:(t + 1) * D], sb_xp[:, t * D:(t + 1) * D],
                    winvp[:, t:t + 1])

            # transposed positives pnt3[ki, ko, m], d = 2*ki+ko, via PE
            with tc.tile_pool(name="psT", bufs=2, space="PSUM") as psT:
                for t in range(NT_P):
                    pst = psT.tile([P, 2, P], bf16, tag="pst", name="pst")
                    xv = xpb.rearrange("p (t d k) -> p t d k", t=NT_P, k=2)
                    for ko in range(2):
                        nc.tensor.transpose(pst[:, ko, :], xv[:, t, :, ko], identb)
                    nc.vector.tensor_copy(pnt3[:, :, t * P:(t + 1) * P], pst)

            # ---- diag dot (fp32 exact): pa_t = <p_i, a_i>
            # own anchor rows live in xa group gc, tiles toff..toff+7 ---
            # emitted late-ish; gated only on xa_raw[gc] load.

            # ---- per-group anchor pipeline + matmul/exp sweep.
            # Emission order pipelines prep one group ahead of the mm/exp
            # sweep so ACT/DVE queues interleave prep(g+1) with exp(g).
            def prep(g):
                # subgroups of 4 tiles pipeline through sumsq -> 1/|a| ->
                # normalize -> sub-transpose so the first matmuls of the
                # group start before the whole group is prepped.
                for s in range(TPG // 4):
                    t0 = s * 4
                    for t in range(t0, t0 + 4):
                        if SUMSQ_ENG[g] == "A":
                            sumsq_act(xa_raw[g], t, ssq_a[g], t)
                        else:
                            sumsq_dve(xa_raw[g], t, ssq_a[g], t)
                    nc.scalar.activation(lns_a[g][:, t0:t0 + 4],
                                         ssq_a[g][:, t0:t0 + 4], AF.Ln)
                    # 1/|a| * KA in one shot: exp(-0.5 ln ssq + ln KA)
                    nc.scalar.activation(inv_a[g][:, t0:t0 + 4],
                                         lns_a[g][:, t0:t0 + 4], AF.Exp,
                                         scale=-0.5, bias=lnka_t[:, 0:1])
                    # normalize + fp8 cast on DVE (2x_2p all-SBUF mode);
                    # gpsimd tensor ops are ~15x slower than spec here.
                    for t in range(t0, t0 + 4):
                        nc.vector.tensor_scalar_mul(
                            a8[g][:, t * D:(t + 1) * D],
                            xa_raw[g][:, t * D:(t + 1) * D],
                            inv_a[g][:, t:t + 1])
                    # pair-transpose the subgroup: bf16 view [j, (4 tiles)]
                    # -> ant[ki, t0:t0+4, j]
                    (nc.sync if g % 2 else nc.scalar).dma_start_transpose(
                        out=ant[g][:, t0:t0 + 4, :],
                        in_=a8[g].bitcast(bf16)[:, t0 * P:(t0 + 4) * P])

            def sweep(g, psM):
                rhs3 = ant[g].bitcast(f8).rearrange(
                    "p t (j k) -> p k (t j)", k=2)
                for m in range(NT_P):
                    ps = psM.tile([P, GCOLS], f32, tag="psmm", name="psmm")
                    for nn in range(GCOLS // NB):
                        nc.tensor.matmul(
                            ps[:, nn * NB:(nn + 1) * NB],
                            pnt3[:, :, m * P:(m + 1) * P],
                            rhs3[:, :, nn * NB:(nn + 1) * NB],
                            start=True, stop=True,
                            perf_mode=MM.DoubleRow,
                        )
                    ucol = m * NGRP + g
                    if (g, m) in DVE_UNITS:
                        ei = ei_pool.tile([P, GCOLS], i16, tag="ei", name="ei")
                        nc.vector.tensor_scalar(
                            out=ei, in0=ps, scalar1=A16, scalar2=B16,
                            op0=ALU.mult, op1=ALU.add)
                        scr2 = exp_pool.tile([P, GCOLS], bf16,
                                             tag="p2scr", name="p2scr")
                        nc.vector.tensor_scalar(
                            out=scr2, in0=ei.bitcast(bf16), scalar1=1.0,
                            scalar2=0.0, op0=ALU.mult, op1=ALU.add,
                            accum_out=ssum[:, ucol:ucol + 1])
                    else:
                        scr = exp_pool.tile([P, GCOLS], f8, tag="expscr",
                                            name="expscr")
                        nc.scalar.activation(
                            scr, ps, AF.Exp, scale=ACT_SCALE,
                            accum_out=ssum[:, ucol:ucol + 1])

            def diag_prep():
                # xa is rotated by c*RPC rows per core, so this core's own
                # anchor rows are exactly xa_raw[0] tiles 0..7, and their
                # KA/|a| inverse norms are inv_a[0][:, 0:8].
                for t in range(NT_P):
                    scr = sq_pool.tile([P, D], f32, tag="sqscr", name="sqscr")
                    nc.vector.scalar_tensor_tensor(
                        out=scr, in0=sb_xp[:, t * D:(t + 1) * D], scalar=1.0,
                        in1=xa_raw[0][:, t * D:(t + 1) * D],
                        op0=ALU.mult, op1=ALU.mult, accum_out=pa[:, t:t + 1],
                    )

            with tc.tile_pool(name="psM", bufs=2, space="PSUM") as psM:
                prep(0)
                prep(1)
                diag_prep()
                sweep(0, psM)
                prep(2)
                sweep(1, psM)
                prep(3)
                sweep(2, psM)
                sweep(3, psM)

            # ---- tail -----------------------------------------------------
            nc.vector.tensor_reduce(
                srow, ssum.rearrange("p (m g) -> p m g", g=NGRP),
                axis=AX.X, op=ALU.add)
            nc.scalar.activation(lnS, srow, AF.Ln)
            # w*cos_ii = pa * inv_p * (inv_a[0][:, 0:8]/KA) * w
            nc.vector.tensor_mul(cosd, pa, inv_p)
            nc.vector.tensor_mul(cosd, cosd, inv_a[0][:, 0:NT_P])
            nc.vector.tensor_scalar_mul(cosd, cosd, float(w) / KA)
            nc.vector.scalar_tensor_tensor(
                out=rowloss, in0=cosd, scalar=-1.0, in1=lnS,
                op0=ALU.mult, op1=ALU.add)
            nc.vector.reduce_sum(rsum, rowloss, axis=AX.X)
            with tc.tile_pool(name="psF", bufs=1, space="PSUM") as psF:
                pfin = psF.tile([1, 1], f32, tag="pfin")
                nc.tensor.matmul(pfin, rsum, ones, start=True, stop=True)
                nc.vector.tensor_copy(sc_out, pfin)
            nc.sync.dma_start(out=out_partial, in_=sc_out)

    nc.compile()
    return nc


def _get_nc(w: float, b: float):
    key = (float(w), float(b))
    if key not in _BUILD_CACHE:
        _BUILD_CACHE[key] = _build(float(w), float(b))
    return _BUILD_CACHE[key]


def build_in_maps(x):
    xa_full = np.ascontiguousarray(x[:, 1, :])
    in_maps = []
    for c in range(NCORES):
        r0 = c * RPC
        # rotate so each core starts streaming at its own shard: spreads
        # the 8 cores' concurrent HBM reads across the whole xa region
        # (they otherwise contend on identical addresses), and makes the
        # core's own anchor rows land in its group-0 tiles (no xad input).
        xa_rot = np.roll(xa_full, -r0, axis=0)
        in_maps.append({
            "xp": np.ascontiguousarray(x[r0:r0 + RPC, 0, :]),
            "xa": np.ascontiguousarray(xa_rot),
        })
    return in_maps


def kernel(x, w, b, epoch=None, **_unused):
    from concourse.bass_utils import run_bass_kernel_spmd

    x = np.asarray(x, dtype=np.float32)
    w_f = float(np.asarray(w))
    b_f = float(np.asarray(b))
    assert x.shape == (N, 2, D), x.shape

    nc = _get_nc(w_f, b_f)
    in_maps = build_in_maps(x)

    res = run_bass_kernel_spmd(nc, in_maps, list(range(NCORES)))
    total = 0.0
    for c in range(NCORES):
        total += float(res.results[c]["partial"][0, 0])
    loss = total / N
    return np.float32(loss)


# revision 19
# speedup vs baseline: 1.1188x; 1.0204x over previous
"""Trainium2 Bass kernel for nn_LossFunction_12532714569881.

Computes, for x: [N=8192, 2, D=256] fp32, w, b scalars:
    P = x[:,0,:]; A = x[:,1,:]
    logits = (P @ A^T) / max(|p_i||a_j|, eps) * w + b        # [N, N]
    loss = -mean_i(log_softmax(logits)[i, i])
The additive b cancels in the row loss: loss_i = ln(sum_j e^{w cos_ij})
- w cos_ii, so no shift is needed anywhere (w cos in [-5, 5] for this
data, no overflow).

Strategy (8 NeuronCores, SPMD, single launch), V2 design:
  - Row-shard: core c owns rows r0=c*1024..+1024. Loads xp (its positive
    block) and xa (full anchors); its own anchor rows are sliced from xa.
  - All scale factors fold into the fp8 operands: positives are scaled
    by w*log2e/(2|p_i|)*kp, anchors by ka/|a_j| (kp=2, ka=8), so the
    matmul psum x satisfies exp(w cos) = 2^(x/8) directly.
  - fp8e4 DoubleRow matmuls (K=256 in one pass, ~1.7x bf16): weights are
    the transposed positives [ki, ko, m] with d=2ki+ko (built via two
    d-strided bf16 PE transposes + DVE copy); the moving operand is the
    anchor matrix pair-transposed IN A SINGLE DMA per 2048-column group:
    normalized fp8 anchors viewed as bf16 pairs, dma_start_transpose to
    [ki, t, j], re-viewed as fp8 [ki, ko(1B), j(2B)] - walrus accepts the
    byte-interleaved moving AP, so the PE never transposes anchors and
    nothing is copied out of PSUM.
  - exp + row-sum of each [128, 2048] psum tile runs on one of two
    engines to break the single-engine exp bottleneck (ACT is 1 elem/
    cycle): ACT units use Exp(scale=ln2/8) with fused accum; DVE units
    use a Schraudolph exponential - one tensor_scalar (x*16 + B -> int16
    = bf16 bits of 2^(x/8)) and one bf16 tensor_reduce at 2 elem/cycle.
    The Schraudolph magic B is calibrated so the loss bias under the
    randn input distribution is ~1e-7 (sensitivity ~7e-4 per 0.01).
  - Anchor norms: sum-of-squares on DVE/ACT (split per group), 1/|a| via
    Ln then Exp(-0.5x + ln ka) on ACT (one shared table set), normalize+
    fp8 cast on gpsimd (plain tensor_scalar; gpsimd accum paths do not
    work in this runtime).
  - The diagonal w*cos_ii is recomputed exactly in fp32 (DVE dots +
    norms), so fp8/Schraudolph noise only perturbs the log-sum-exp where
    it averages out. Expected rel err ~1e-4 (gate 2e-2).
  - Each core emits one partial scalar = sum of its 1024 row losses;
    the host sums and divides by N.
"""

import numpy as np

N = 8192
D = 256
NCORES = 8
RPC = N // NCORES          # 1024 rows per core
P = 128                    # partitions
NT_P = RPC // P            # 8 positive tiles / m-chunks
GCOLS = 2048               # columns per group
NGRP = N // GCOLS          # 4 column groups
TPG = GCOLS // P           # 16 anchor tiles per group
NB = 512                   # moving j-slice per DR matmul

LOG2E = 1.4426950408889634
KP = 2.0                   # positive fp8 scale headroom
KA = 8.0                   # anchor fp8 scale
SCHRA_C = 0.057101         # Schraudolph bias constant (calibrated)
A16 = 2.0 ** 23 / 8.0 / 65536.0          # = 16.0
B16 = (127.0 - SCHRA_C) * 128.0          # int16 magic
ACT_SCALE = float(np.log(2.0) / 8.0)     # exp(x*ln2/8) = 2^(x/8)

# exp unit assignment: units are (g, m); 'D' units run Schraudolph on DVE.
# Tuned for ACT/DVE balance (ACT ~2.0us/unit incl overhead, DVE ~3.4us).
DVE_UNITS = {(g, 3) for g in range(NGRP)} | {(2, 6), (3, 6)}
# sum-of-squares engine per anchor group: 'A' (ACT Square) or 'D' (DVE stt)
SUMSQ_ENG = ["A", "D", "D", "D"]

_BUILD_CACHE = {}
_ACT_TABLES_PATCHED = False
_LDW_OPT_PATCHED = False


def _patch_ldw_opt():
    """Enable walrus's redundant-LDWEIGHTS elision (hardcoded off in
    bass_utils); consecutive same-weight matmuls (our nn-runs of 4) then
    skip the PE array reload."""
    global _LDW_OPT_PATCHED
    if _LDW_OPT_PATCHED:
        return
    import concourse.bass_utils as bu

    orig_run = bu.run_command

    def patched(argv, **kwargs):
        argv = [a.replace("--enable-ldw-opt=false", "--enable-ldw-opt=true")
                if isinstance(a, str) else a for a in argv]
        return orig_run(argv, **kwargs)

    bu.run_command = patched
    _LDW_OPT_PATCHED = True


def _patch_act_tables():
    """Make Exp and Ln resolve to the one table set containing both, so a
    single ACT_TABLE_LOAD serves the whole kernel."""
    global _ACT_TABLES_PATCHED
    if _ACT_TABLES_PATCHED:
        return
    import concourse.bacc as bacc_mod
    import concourse.bass_interp as interp_mod
    import concourse.mybir as mybir
    from concourse import hw_specs

    AF = mybir.ActivationFunctionType
    orig = hw_specs.get_activation_tables

    def patched(module_arch):
        tabs = orig(module_arch)
        out = {}
        for name, funcs in tabs.items():
            f = set(funcs)
            if name != "natural_log_exp_and_others":
                f.discard(AF.Exp)
                f.discard(AF.Ln)
            out[name] = f
        return out

    bacc_mod.get_activation_tables = patched
    interp_mod.get_activation_tables = patched
    _ACT_TABLES_PATCHED = True


def _build(w: float, b: float):
    from contextlib import ExitStack

    import concourse.bass as bass  # noqa: F401
    import concourse.mybir as mybir
    import concourse.tile as tile
    from concourse import bacc

    _patch_act_tables()

    f32 = mybir.dt.float32
    f8 = mybir.dt.float8e4
    bf16 = mybir.dt.bfloat16
    i16 = mybir.dt.int16
    AF = mybir.ActivationFunctionType
    ALU = mybir.AluOpType
    AX = mybir.AxisListType
    MM = mybir.MatmulPerfMode

    nc = bacc.Bacc("TRN2", target_bir_lowering=False, debug=False)

    xp = nc.dram_tensor("xp", [RPC, D], f32, kind="ExternalInput").ap()
    xa = nc.dram_tensor("xa", [N, D], f32, kind="ExternalInput").ap()
    out_partial = nc.dram_tensor("partial", [1, 1], f32, kind="ExternalOutput").ap()

    with tile.TileContext(nc) as tc:
        with ExitStack() as ctx:
            sing = ctx.enter_context(tc.tile_pool(name="sing", bufs=1))
            sq_pool = ctx.enter_context(tc.tile_pool(name="sqp", bufs=3))
            exp_pool = ctx.enter_context(tc.tile_pool(name="expp", bufs=2))
            ei_pool = ctx.enter_context(tc.tile_pool(name="eip", bufs=2))

            # ---- persistent SBUF tensors
            xa_raw = [sing.tile([P, TPG * D], f32, tag=f"xar{g}", name=f"xar{g}")
                      for g in range(NGRP)]
            a8 = [sing.tile([P, TPG * D], f8, tag=f"a8_{g}", name=f"a8_{g}")
                  for g in range(NGRP)]
            ant = [sing.tile([P, TPG, P], bf16, tag=f"ant{g}", name=f"ant{g}")
                   for g in range(NGRP)]
            ssq_a = [sing.tile([P, TPG], f32, tag=f"ssqa{g}", name=f"ssqa{g}")
                     for g in range(NGRP)]
            lns_a = [sing.tile([P, TPG], f32, tag=f"lnsa{g}", name=f"lnsa{g}")
                     for g in range(NGRP)]
            inv_a = [sing.tile([P, TPG], f32, tag=f"inva{g}", name=f"inva{g}")
                     for g in range(NGRP)]

            sb_xp = sing.tile([P, NT_P * D], f32, tag="xp")
            xpb = sing.tile([P, NT_P * D], bf16, tag="xpb")
            pnt3 = sing.tile([P, 2, RPC], f8, tag="pnt3")
            identb = sing.tile([P, P], bf16, tag="identb")

            ssq_p = sing.tile([P, NT_P], f32, tag="ssqp")
            lns_p = sing.tile([P, NT_P], f32, tag="lnsp")
            inv_p = sing.tile([P, NT_P], f32, tag="invp")
            winvp = sing.tile([P, NT_P], f32, tag="winvp")
            pa = sing.tile([P, NT_P], f32, tag="pa")
            ssum = sing.tile([P, NT_P * NGRP], f32, tag="ssum")
            srow = sing.tile([P, NT_P], f32, tag="srow")
            lnS = sing.tile([P, NT_P], f32, tag="lnS")
            cosd = sing.tile([P, NT_P], f32, tag="cosd")
            rowloss = sing.tile([P, NT_P], f32, tag="rowloss")
            rsum = sing.tile([P, 1], f32, tag="rsum")
            ones = sing.tile([P, 1], f32, tag="ones")
            lnka_t = sing.tile([P, 1], f32, tag="lnka")
            sc_out = sing.tile([1, 1], f32, tag="sc_out")

            from concourse.masks import make_identity
            make_identity(nc, identb[:])
            nc.vector.memset(ones, 1.0)
            nc.vector.memset(lnka_t, float(np.log(KA)))

            # ---- loads: fine-grained chunks so compute starts early;
            # gpsimd's DMA queue stalls the machine - avoid it entirely.
            # Order: xp + xad first (gate P-prep/diag), then xa groups in
            # ascending order, each group split across both queues.
            # DMA issues share the engine instruction queues; keep the
            # early-critical loads (xp, xad, g0, g1) up front on sync/
            # scalar, and push g2/g3 loads to gpsimd (idle; slower pacing
            # is fine since that data is needed only ~60us in).
            # anchor row j (within rotated xa) <-> group g = j // 2048,
            # half h = (j % 2048) // 1024, partition p = (j % 1024) // 8,
            # u = j % 8; tile index inside the group is t = h*8 + u, so the
            # first 1024 rows (this core's own anchors) are tiles 0..7 and
            # pair up with the positives partition-for-partition.
            xa_v = xa.rearrange("(g h p u) d -> p g h u d", g=NGRP, h=2, p=P)

            def load_group(g, eng):
                for h in range(2):
                    for u0 in (0, 4):
                        eng.dma_start(
                            out=xa_raw[g].rearrange(
                                "p (h u d) -> p h u d", h=2, u=8)[:, h, u0:u0 + 4, :],
                            in_=xa_v[:, g, h, u0:u0 + 4, :],
                        )

            # Per-queue DMA bandwidth is ~150GB/s, so the early-critical
            # data (xp, g0) is split across the scalar+sync queues while
            # later groups ride the vector/tensor/gpsimd queues (idle at
            # t=0; their issue cost there is negligible).
            # row i <-> (p, t) = (i // 8, i % 8): each partition reads
            # contiguous DRAM (4KB per chunk-descriptor instead of 1KB,
            # which was descriptor-rate-bound at ~50GB/s).
            for h in range(2):
                nc.scalar.dma_start(
                    out=sb_xp.rearrange("p (t d) -> p t d", d=D)[:, h * 4:(h + 1) * 4, :],
                    in_=xp.rearrange("(p t) d -> p t d", p=P)[:, h * 4:(h + 1) * 4, :],
                )
            for ci, (h, u0) in enumerate(((0, 0), (0, 4), (1, 0), (1, 4))):
                (nc.sync if ci < 2 else nc.scalar).dma_start(
                    out=xa_raw[0].rearrange(
                        "p (h u d) -> p h u d", h=2, u=8)[:, h, u0:u0 + 4, :],
                    in_=xa_v[:, 0, h, u0:u0 + 4, :],
                )
            load_group(1, nc.sync)
            load_group(2, nc.gpsimd)
            load_group(3, nc.gpsimd)

            def sumsq_dve(src, t, acc, col):
                scr = sq_pool.tile([P, D], f32, tag="sqscr", name="sqscr")
                nc.vector.scalar_tensor_tensor(
                    out=scr, in0=src[:, t * D:(t + 1) * D], scalar=1.0,
                    in1=src[:, t * D:(t + 1) * D],
                    op0=ALU.mult, op1=ALU.mult, accum_out=acc[:, col:col + 1],
                )

            def sumsq_act(src, t, acc, col):
                scr = sq_pool.tile([P, D], f32, tag="asqscr", name="asqscr")
                nc.scalar.activation(
                    scr, src[:, t * D:(t + 1) * D], AF.Square,
                    accum_out=acc[:, col:col + 1],
                )

            # ---- P-side prep: norms (fp32-exact), scaled bf16+fp8 casts
            for t in range(NT_P):
                sumsq_dve(sb_xp, t, ssq_p, t)
            nc.scalar.activation(lns_p, ssq_p, AF.Ln)
            nc.scalar.activation(inv_p, lns_p, AF.Exp, scale=-0.5)
            nc.vector.tensor_scalar_mul(winvp, inv_p, float(w) * LOG2E / 2.0 * KP)
            for t in range(NT_P):
                nc.vector.tensor_scalar_mul(
                    xpb[:, t * D:(t + 1) * D], sb_xp[:, t * D:(t + 1) * D],
                    winvp[:, t:t + 1])

            # transposed positives pnt3[ki, ko, m], d = 2*ki+ko, via PE
            with tc.tile_pool(name="psT", bufs=2, space="PSUM") as psT:
                for t in range(NT_P):
                    pst = psT.tile([P, 2, P], bf16, tag="pst", name="pst")
                    xv = xpb.rearrange("p (t d k) -> p t d k", t=NT_P, k=2)
                    for ko in range(2):
                        nc.tensor.transpose(pst[:, ko, :], xv[:, t, :, ko], identb)
                    nc.vector.tensor_copy(pnt3[:, :, t * P:(t + 1) * P], pst)

            # ---- diag dot (fp32 exact): pa_t = <p_i, a_i>
            # own anchor rows live in xa group gc, tiles toff..toff+7 ---
            # emitted late-ish; gated only on xa_raw[gc] load.

            # ---- per-group anchor pipeline + matmul/exp sweep.
            # Emission order pipelines prep one group ahead of the mm/exp
            # sweep so ACT/DVE queues interleave prep(g+1) with exp(g).
            def prep(g):
                # subgroups of 4 tiles pipeline through sumsq -> 1/|a| ->
                # normalize -> sub-transpose so the first matmuls of the
                # group start before the whole group is prepped.
                for s in range(TPG // 4):
                    t0 = s * 4
                    for t in range(t0, t0 + 4):
                        if SUMSQ_ENG[g] == "A":
                            sumsq_act(xa_raw[g], t, ssq_a[g], t)
                        else:
                            sumsq_dve(xa_raw[g], t, ssq_a[g], t)
                    nc.scalar.activation(lns_a[g][:, t0:t0 + 4],
                                         ssq_a[g][:, t0:t0 + 4], AF.Ln)
                    # 1/|a| * KA in one shot: exp(-0.5 ln ssq + ln KA)
                    nc.scalar.activation(inv_a[g][:, t0:t0 + 4],
                                         lns_a[g][:, t0:t0 + 4], AF.Exp,
                                         scale=-0.5, bias=lnka_t[:, 0:1])
                    # normalize + fp8 cast on DVE (2x_2p all-SBUF mode);
                    # gpsimd tensor ops are ~15x slower than spec here.
                    for t in range(t0, t0 + 4):
                        nc.vector.tensor_scalar_mul(
                            a8[g][:, t * D:(t + 1) * D],
                            xa_raw[g][:, t * D:(t + 1) * D],
                            inv_a[g][:, t:t + 1])
                    # pair-transpose the subgroup: bf16 view [j, (4 tiles)]
                    # -> ant[ki, t0:t0+4, j]
                    (nc.sync if g % 2 else nc.scalar).dma_start_transpose(
                        out=ant[g][:, t0:t0 + 4, :],
                        in_=a8[g].bitcast(bf16)[:, t0 * P:(t0 + 4) * P])

            def sweep(g, psM):
                rhs3 = ant[g].bitcast(f8).rearrange(
                    "p t (j k) -> p k (t j)", k=2)
                for m in range(NT_P):
                    ps = psM.tile([P, GCOLS], f32, tag="psmm", name="psmm")
                    for nn in range(GCOLS // NB):
                        nc.tensor.matmul(
                            ps[:, nn * NB:(nn + 1) * NB],
                            pnt3[:, :, m * P:(m + 1) * P],
                            rhs3[:, :, nn * NB:(nn + 1) * NB],
                            start=True, stop=True,
                            perf_mode=MM.DoubleRow,
                        )
                    ucol = m * NGRP + g
                    if (g, m) in DVE_UNITS:
                        ei = ei_pool.tile([P, GCOLS], i16, tag="ei", name="ei")
                        nc.vector.tensor_scalar(
                            out=ei, in0=ps, scalar1=A16, scalar2=B16,
                            op0=ALU.mult, op1=ALU.add)
                        scr2 = exp_pool.tile([P, GCOLS], bf16,
                                             tag="p2scr", name="p2scr")
                        nc.vector.tensor_scalar(
                            out=scr2, in0=ei.bitcast(bf16), scalar1=1.0,
                            scalar2=0.0, op0=ALU.mult, op1=ALU.add,
                            accum_out=ssum[:, ucol:ucol + 1])
                    else:
                        scr = exp_pool.tile([P, GCOLS], f8, tag="expscr",
                                            name="expscr")
                        nc.scalar.activation(
                            scr, ps, AF.Exp, scale=ACT_SCALE,
                            accum_out=ssum[:, ucol:ucol + 1])

            def diag_prep():
                # xa is rotated by c*RPC rows per core, so this core's own
                # anchor rows are exactly xa_raw[0] tiles 0..7, and their
                # KA/|a| inverse norms are inv_a[0][:, 0:8].
                for t in range(NT_P):
                    scr = sq_pool.tile([P, D], f32, tag="sqscr", name="sqscr")
                    nc.vector.scalar_tensor_tensor(
                        out=scr, in0=sb_xp[:, t * D:(t + 1) * D], scalar=1.0,
                        in1=xa_raw[0][:, t * D:(t + 1) * D],
                        op0=ALU.mult, op1=ALU.mult, accum_out=pa[:, t:t + 1],
                    )

            with tc.tile_pool(name="psM", bufs=2, space="PSUM") as psM:
                prep(0)
                prep(1)
                diag_prep()
                sweep(0, psM)
                prep(2)
                sweep(1, psM)
                prep(3)
                sweep(2, psM)
                sweep(3, psM)

            # ---- tail -----------------------------------------------------
            nc.vector.tensor_reduce(
                srow, ssum.rearrange("p (m g) -> p m g", g=NGRP),
                axis=AX.X, op=ALU.add)
            nc.scalar.activation(lnS, srow, AF.Ln)
            # w*cos_ii = pa * inv_p * (inv_a[0][:, 0:8]/KA) * w
            nc.vector.tensor_mul(cosd, pa, inv_p)
            nc.vector.tensor_mul(cosd, cosd, inv_a[0][:, 0:NT_P])
            nc.vector.tensor_scalar_mul(cosd, cosd, float(w) / KA)
            nc.vector.scalar_tensor_tensor(
                out=rowloss, in0=cosd, scalar=-1.0, in1=lnS,
                op0=ALU.mult, op1=ALU.add)
            nc.vector.reduce_sum(rsum, rowloss, axis=AX.X)
            with tc.tile_pool(name="psF", bufs=1, space="PSUM") as psF:
                pfin = psF.tile([1, 1], f32, tag="pfin")
                nc.tensor.matmul(pfin, rsum, ones, start=True, stop=True)
                nc.vector.tensor_copy(sc_out, pfin)
            nc.sync.dma_start(out=out_partial, in_=sc_out)

    nc.compile()
    return nc


def _get_nc(w: float, b: float):
    key = (float(w), float(b))
    if key not in _BUILD_CACHE:
        _BUILD_CACHE[key] = _build(float(w), float(b))
    return _BUILD_CACHE[key]


def build_in_maps(x):
    xa_full = np.ascontiguousarray(x[:, 1, :])
    in_maps = []
    for c in range(NCORES):
        r0 = c * RPC
        # rotate so each core starts streaming at its own shard: spreads
        # the 8 cores' concurrent HBM reads across the whole xa region
        # (they otherwise contend on identical addresses), and makes the
        # core's own anchor rows land in its group-0 tiles (no xad input).
        xa_rot = np.roll(xa_full, -r0, axis=0)
        in_maps.append({
            "xp": np.ascontiguousarray(x[r0:r0 + RPC, 0, :]),
            "xa": np.ascontiguousarray(xa_rot),
        })
    return in_maps


def kernel(x, w, b, epoch=None, **_unused):
    from concourse.bass_utils import run_bass_kernel_spmd

    x = np.asarray(x, dtype=np.float32)
    w_f = float(np.asarray(w))
    b_f = float(np.asarray(b))
    assert x.shape == (N, 2, D), x.shape

    nc = _get_nc(w_f, b_f)
    in_maps = build_in_maps(x)

    res = run_bass_kernel_spmd(nc, in_maps, list(range(NCORES)))
    total = 0.0
    for c in range(NCORES):
        total += float(res.results[c]["partial"][0, 0])
    loss = total / N
    return np.float32(loss)


# revision 20
# speedup vs baseline: 1.1289x; 1.0090x over previous
"""Trainium2 Bass kernel for nn_LossFunction_12532714569881.

Computes, for x: [N=8192, 2, D=256] fp32, w, b scalars:
    P = x[:,0,:]; A = x[:,1,:]
    logits = (P @ A^T) / max(|p_i||a_j|, eps) * w + b        # [N, N]
    loss = -mean_i(log_softmax(logits)[i, i])
The additive b cancels in the row loss: loss_i = ln(sum_j e^{w cos_ij})
- w cos_ii, so no shift is needed anywhere (w cos in [-5, 5] for this
data, no overflow).

Strategy (8 NeuronCores, SPMD, single launch), V2 design:
  - Row-shard: core c owns rows r0=c*1024..+1024. Loads xp (its positive
    block) and xa (full anchors); its own anchor rows are sliced from xa.
  - All scale factors fold into the fp8 operands: positives are scaled
    by w*log2e/(2|p_i|)*kp, anchors by ka/|a_j| (kp=2, ka=8), so the
    matmul psum x satisfies exp(w cos) = 2^(x/8) directly.
  - fp8e4 DoubleRow matmuls (K=256 in one pass, ~1.7x bf16): weights are
    the transposed positives [ki, ko, m] with d=2ki+ko (built via two
    d-strided bf16 PE transposes + DVE copy); the moving operand is the
    anchor matrix pair-transposed IN A SINGLE DMA per 2048-column group:
    normalized fp8 anchors viewed as bf16 pairs, dma_start_transpose to
    [ki, t, j], re-viewed as fp8 [ki, ko(1B), j(2B)] - walrus accepts the
    byte-interleaved moving AP, so the PE never transposes anchors and
    nothing is copied out of PSUM.
  - exp + row-sum of each [128, 2048] psum tile runs on one of two
    engines to break the single-engine exp bottleneck (ACT is 1 elem/
    cycle): ACT units use Exp(scale=ln2/8) with fused accum; DVE units
    use a Schraudolph exponential - one tensor_scalar (x*16 + B -> int16
    = bf16 bits of 2^(x/8)) and one bf16 tensor_reduce at 2 elem/cycle.
    The Schraudolph magic B is calibrated so the loss bias under the
    randn input distribution is ~1e-7 (sensitivity ~7e-4 per 0.01).
  - Anchor norms: sum-of-squares on DVE/ACT (split per group), 1/|a| via
    Ln then Exp(-0.5x + ln ka) on ACT (one shared table set), normalize+
    fp8 cast on gpsimd (plain tensor_scalar; gpsimd accum paths do not
    work in this runtime).
  - The diagonal w*cos_ii is recomputed exactly in fp32 (DVE dots +
    norms), so fp8/Schraudolph noise only perturbs the log-sum-exp where
    it averages out. Expected rel err ~1e-4 (gate 2e-2).
  - Each core emits one partial scalar = sum of its 1024 row losses;
    the host sums and divides by N.
"""

import numpy as np

N = 8192
D = 256
NCORES = 8
RPC = N // NCORES          # 1024 rows per core
P = 128                    # partitions
NT_P = RPC // P            # 8 positive tiles / m-chunks
GCOLS = 2048               # columns per group
NGRP = N // GCOLS          # 4 column groups
TPG = GCOLS // P           # 16 anchor tiles per group
NB = 512                   # moving j-slice per DR matmul

LOG2E = 1.4426950408889634
KP = 2.0                   # positive fp8 scale headroom
KA = 8.0                   # anchor fp8 scale
SCHRA_C = 0.057101         # Schraudolph bias constant (calibrated)
A16 = 2.0 ** 23 / 8.0 / 65536.0          # = 16.0
B16 = (127.0 - SCHRA_C) * 128.0          # int16 magic
ACT_SCALE = float(np.log(2.0) / 8.0)     # exp(x*ln2/8) = 2^(x/8)

# exp unit assignment: units are (g, m); 'D' units run Schraudolph on DVE.
# Tuned for ACT/DVE balance (ACT ~2.0us/unit incl overhead, DVE ~3.4us).
DVE_UNITS = {(g, 3) for g in range(NGRP)} | {(1, 6), (2, 6), (3, 6)}
# sum-of-squares engine per anchor group: 'A' (ACT Square) or 'D' (DVE stt)
SUMSQ_ENG = ["A", "D", "D", "D"]

_BUILD_CACHE = {}
_ACT_TABLES_PATCHED = False
_LDW_OPT_PATCHED = False


def _patch_ldw_opt():
    """Enable walrus's redundant-LDWEIGHTS elision (hardcoded off in
    bass_utils); consecutive same-weight matmuls (our nn-runs of 4) then
    skip the PE array reload."""
    global _LDW_OPT_PATCHED
    if _LDW_OPT_PATCHED:
        return
    import concourse.bass_utils as bu

    orig_run = bu.run_command

    def patched(argv, **kwargs):
        argv = [a.replace("--enable-ldw-opt=false", "--enable-ldw-opt=true")
                if isinstance(a, str) else a for a in argv]
        return orig_run(argv, **kwargs)

    bu.run_command = patched
    _LDW_OPT_PATCHED = True


def _patch_act_tables():
    """Make Exp and Ln resolve to the one table set containing both, so a
    single ACT_TABLE_LOAD serves the whole kernel."""
    global _ACT_TABLES_PATCHED
    if _ACT_TABLES_PATCHED:
        return
    import concourse.bacc as bacc_mod
    import concourse.bass_interp as interp_mod
    import concourse.mybir as mybir
    from concourse import hw_specs

    AF = mybir.ActivationFunctionType
    orig = hw_specs.get_activation_tables

    def patched(module_arch):
        tabs = orig(module_arch)
        out = {}
        for name, funcs in tabs.items():
            f = set(funcs)
            if name != "natural_log_exp_and_others":
                f.discard(AF.Exp)
                f.discard(AF.Ln)
            out[name] = f
        return out

    bacc_mod.get_activation_tables = patched
    interp_mod.get_activation_tables = patched
    _ACT_TABLES_PATCHED = True


def _build(w: float, b: float):
    from contextlib import ExitStack

    import concourse.bass as bass  # noqa: F401
    import concourse.mybir as mybir
    import concourse.tile as tile
    from concourse import bacc

    _patch_act_tables()

    f32 = mybir.dt.float32
    f8 = mybir.dt.float8e4
    bf16 = mybir.dt.bfloat16
    i16 = mybir.dt.int16
    AF = mybir.ActivationFunctionType
    ALU = mybir.AluOpType
    AX = mybir.AxisListType
    MM = mybir.MatmulPerfMode

    nc = bacc.Bacc("TRN2", target_bir_lowering=False, debug=False)

    xp = nc.dram_tensor("xp", [RPC, D], f32, kind="ExternalInput").ap()
    xa = nc.dram_tensor("xa", [N, D], f32, kind="ExternalInput").ap()
    out_partial = nc.dram_tensor("partial", [1, 1], f32, kind="ExternalOutput").ap()

    with tile.TileContext(nc) as tc:
        with ExitStack() as ctx:
            sing = ctx.enter_context(tc.tile_pool(name="sing", bufs=1))
            sq_pool = ctx.enter_context(tc.tile_pool(name="sqp", bufs=3))
            exp_pool = ctx.enter_context(tc.tile_pool(name="expp", bufs=2))
            ei_pool = ctx.enter_context(tc.tile_pool(name="eip", bufs=2))

            # ---- persistent SBUF tensors
            xa_raw = [sing.tile([P, TPG * D], f32, tag=f"xar{g}", name=f"xar{g}")
                      for g in range(NGRP)]
            a8 = [sing.tile([P, TPG * D], f8, tag=f"a8_{g}", name=f"a8_{g}")
                  for g in range(NGRP)]
            ant = [sing.tile([P, TPG, P], bf16, tag=f"ant{g}", name=f"ant{g}")
                   for g in range(NGRP)]
            ssq_a = [sing.tile([P, TPG], f32, tag=f"ssqa{g}", name=f"ssqa{g}")
                     for g in range(NGRP)]
            lns_a = [sing.tile([P, TPG], f32, tag=f"lnsa{g}", name=f"lnsa{g}")
                     for g in range(NGRP)]
            inv_a = [sing.tile([P, TPG], f32, tag=f"inva{g}", name=f"inva{g}")
                     for g in range(NGRP)]

            sb_xp = sing.tile([P, NT_P * D], f32, tag="xp")
            xpb = sing.tile([P, NT_P * D], bf16, tag="xpb")
            pnt3 = sing.tile([P, 2, RPC], f8, tag="pnt3")
            identb = sing.tile([P, P], bf16, tag="identb")

            ssq_p = sing.tile([P, NT_P], f32, tag="ssqp")
            lns_p = sing.tile([P, NT_P], f32, tag="lnsp")
            inv_p = sing.tile([P, NT_P], f32, tag="invp")
            winvp = sing.tile([P, NT_P], f32, tag="winvp")
            pa = sing.tile([P, NT_P], f32, tag="pa")
            ssum = sing.tile([P, NT_P * NGRP], f32, tag="ssum")
            srow = sing.tile([P, NT_P], f32, tag="srow")
            lnS = sing.tile([P, NT_P], f32, tag="lnS")
            cosd = sing.tile([P, NT_P], f32, tag="cosd")
            rowloss = sing.tile([P, NT_P], f32, tag="rowloss")
            rsum = sing.tile([P, 1], f32, tag="rsum")
            ones = sing.tile([P, 1], f32, tag="ones")
            lnka_t = sing.tile([P, 1], f32, tag="lnka")
            sc_out = sing.tile([1, 1], f32, tag="sc_out")

            from concourse.masks import make_identity
            make_identity(nc, identb[:])
            nc.vector.memset(ones, 1.0)
            nc.vector.memset(lnka_t, float(np.log(KA)))

            # ---- loads: fine-grained chunks so compute starts early;
            # gpsimd's DMA queue stalls the machine - avoid it entirely.
            # Order: xp + xad first (gate P-prep/diag), then xa groups in
            # ascending order, each group split across both queues.
            # DMA issues share the engine instruction queues; keep the
            # early-critical loads (xp, xad, g0, g1) up front on sync/
            # scalar, and push g2/g3 loads to gpsimd (idle; slower pacing
            # is fine since that data is needed only ~60us in).
            # anchor row j (within rotated xa) <-> group g = j // 2048,
            # half h = (j % 2048) // 1024, partition p = (j % 1024) // 8,
            # u = j % 8; tile index inside the group is t = h*8 + u, so the
            # first 1024 rows (this core's own anchors) are tiles 0..7 and
            # pair up with the positives partition-for-partition.
            xa_v = xa.rearrange("(g h p u) d -> p g h u d", g=NGRP, h=2, p=P)

            def load_group(g, eng):
                for h in range(2):
                    eng.dma_start(
                        out=xa_raw[g].rearrange(
                            "p (h u d) -> p h u d", h=2, u=8)[:, h, :, :],
                        in_=xa_v[:, g, h, :, :],
                    )

            # Per-queue DMA bandwidth is ~150GB/s, so the early-critical
            # data (xp, g0) is split across the scalar+sync queues while
            # later groups ride the vector/tensor/gpsimd queues (idle at
            # t=0; their issue cost there is negligible).
            # row i <-> (p, t) = (i // 8, i % 8): each partition reads
            # contiguous DRAM (4KB per chunk-descriptor instead of 1KB,
            # which was descriptor-rate-bound at ~50GB/s).
            for h in range(2):
                nc.gpsimd.dma_start(
                    out=sb_xp.rearrange("p (t d) -> p t d", d=D)[:, h * 4:(h + 1) * 4, :],
                    in_=xp.rearrange("(p t) d -> p t d", p=P)[:, h * 4:(h + 1) * 4, :],
                )
            for ci, (h, u0) in enumerate(((0, 0), (1, 0), (0, 4), (1, 4))):
                (nc.sync if ci % 2 == 0 else nc.scalar).dma_start(
                    out=xa_raw[0].rearrange(
                        "p (h u d) -> p h u d", h=2, u=8)[:, h, u0:u0 + 4, :],
                    in_=xa_v[:, 0, h, u0:u0 + 4, :],
                )
            load_group(1, nc.sync)
            load_group(2, nc.gpsimd)
            load_group(3, nc.gpsimd)

            def sumsq_dve(src, t, acc, col):
                scr = sq_pool.tile([P, D], f32, tag="sqscr", name="sqscr")
                nc.vector.scalar_tensor_tensor(
                    out=scr, in0=src[:, t * D:(t + 1) * D], scalar=1.0,
                    in1=src[:, t * D:(t + 1) * D],
                    op0=ALU.mult, op1=ALU.mult, accum_out=acc[:, col:col + 1],
                )

            def sumsq_act(src, t, acc, col):
                scr = sq_pool.tile([P, D], f32, tag="asqscr", name="asqscr")
                nc.scalar.activation(
                    scr, src[:, t * D:(t + 1) * D], AF.Square,
                    accum_out=acc[:, col:col + 1],
                )

            # ---- P-side prep: norms (fp32-exact), scaled bf16+fp8 casts
            for t in range(NT_P):
                sumsq_dve(sb_xp, t, ssq_p, t)
            nc.scalar.activation(lns_p, ssq_p, AF.Ln)
            nc.scalar.activation(inv_p, lns_p, AF.Exp, scale=-0.5)
            nc.vector.tensor_scalar_mul(winvp, inv_p, float(w) * LOG2E / 2.0 * KP)
            for t in range(NT_P):
                nc.vector.tensor_scalar_mul(
                    xpb[:, t * D:(t + 1) * D], sb_xp[:, t * D:(t + 1) * D],
                    winvp[:, t:t + 1])

            # transposed positives pnt3[ki, ko, m], d = 2*ki+ko, via PE
            with tc.tile_pool(name="psT", bufs=2, space="PSUM") as psT:
                for t in range(NT_P):
                    pst = psT.tile([P, 2, P], bf16, tag="pst", name="pst")
                    xv = xpb.rearrange("p (t d k) -> p t d k", t=NT_P, k=2)
                    for ko in range(2):
                        nc.tensor.transpose(pst[:, ko, :], xv[:, t, :, ko], identb)
                    nc.vector.tensor_copy(pnt3[:, :, t * P:(t + 1) * P], pst)

            # ---- diag dot (fp32 exact): pa_t = <p_i, a_i>
            # own anchor rows live in xa group gc, tiles toff..toff+7 ---
            # emitted late-ish; gated only on xa_raw[gc] load.

            # ---- per-group anchor pipeline + matmul/exp sweep.
            # Emission order pipelines prep one group ahead of the mm/exp
            # sweep so ACT/DVE queues interleave prep(g+1) with exp(g).
            def prep(g):
                # subgroups of 4 tiles pipeline through sumsq -> 1/|a| ->
                # normalize -> sub-transpose so the first matmuls of the
                # group start before the whole group is prepped.
                for s in range(TPG // 4):
                    t0 = s * 4
                    for t in range(t0, t0 + 4):
                        if SUMSQ_ENG[g] == "A":
                            sumsq_act(xa_raw[g], t, ssq_a[g], t)
                        else:
                            sumsq_dve(xa_raw[g], t, ssq_a[g], t)
                    nc.scalar.activation(lns_a[g][:, t0:t0 + 4],
                                         ssq_a[g][:, t0:t0 + 4], AF.Ln)
                    # 1/|a| * KA in one shot: exp(-0.5 ln ssq + ln KA)
                    nc.scalar.activation(inv_a[g][:, t0:t0 + 4],
                                         lns_a[g][:, t0:t0 + 4], AF.Exp,
                                         scale=-0.5, bias=lnka_t[:, 0:1])
                    # normalize + fp8 cast on DVE (2x_2p all-SBUF mode);
                    # gpsimd tensor ops are ~15x slower than spec here.
                    for t in range(t0, t0 + 4):
                        nc.vector.tensor_scalar_mul(
                            a8[g][:, t * D:(t + 1) * D],
                            xa_raw[g][:, t * D:(t + 1) * D],
                            inv_a[g][:, t:t + 1])
                    # pair-transpose the subgroup: bf16 view [j, (4 tiles)]
                    # -> ant[ki, t0:t0+4, j]
                    (nc.sync if g % 2 else nc.scalar).dma_start_transpose(
                        out=ant[g][:, t0:t0 + 4, :],
                        in_=a8[g].bitcast(bf16)[:, t0 * P:(t0 + 4) * P])

            def sweep(g, psM):
                rhs3 = ant[g].bitcast(f8).rearrange(
                    "p t (j k) -> p k (t j)", k=2)
                for m in range(NT_P):
                    ps = psM.tile([P, GCOLS], f32, tag="psmm", name="psmm")
                    for nn in range(GCOLS // NB):
                        nc.tensor.matmul(
                            ps[:, nn * NB:(nn + 1) * NB],
                            pnt3[:, :, m * P:(m + 1) * P],
                            rhs3[:, :, nn * NB:(nn + 1) * NB],
                            start=True, stop=True,
                            perf_mode=MM.DoubleRow,
                        )
                    ucol = m * NGRP + g
                    if (g, m) in DVE_UNITS:
                        ei = ei_pool.tile([P, GCOLS], i16, tag="ei", name="ei")
                        nc.vector.tensor_scalar(
                            out=ei, in0=ps, scalar1=A16, scalar2=B16,
                            op0=ALU.mult, op1=ALU.add)
                        scr2 = exp_pool.tile([P, GCOLS], bf16,
                                             tag="p2scr", name="p2scr")
                        nc.vector.tensor_scalar(
                            out=scr2, in0=ei.bitcast(bf16), scalar1=1.0,
                            scalar2=0.0, op0=ALU.mult, op1=ALU.add,
                            accum_out=ssum[:, ucol:ucol + 1])
                    else:
                        scr = exp_pool.tile([P, GCOLS], f8, tag="expscr",
                                            name="expscr")
                        nc.scalar.activation(
                            scr, ps, AF.Exp, scale=ACT_SCALE,
                            accum_out=ssum[:, ucol:ucol + 1])

            def diag_prep():
                # xa is rotated by c*RPC rows per core, so this core's own
                # anchor rows are exactly xa_raw[0] tiles 0..7, and their
                # KA/|a| inverse norms are inv_a[0][:, 0:8].
                for t in range(NT_P):
                    scr = sq_pool.tile([P, D], f32, tag="sqscr", name="sqscr")
                    nc.vector.scalar_tensor_tensor(
                        out=scr, in0=sb_xp[:, t * D:(t + 1) * D], scalar=1.0,
                        in1=xa_raw[0][:, t * D:(t + 1) * D],
                        op0=ALU.mult, op1=ALU.mult, accum_out=pa[:, t:t + 1],
                    )

            with tc.tile_pool(name="psM", bufs=2, space="PSUM") as psM:
                prep(0)
                prep(1)
                diag_prep()
                sweep(0, psM)
                prep(2)
                sweep(1, psM)
                prep(3)
                sweep(2, psM)
                sweep(3, psM)

            # ---- tail -----------------------------------------------------
            nc.vector.tensor_reduce(
                srow, ssum.rearrange("p (m g) -> p m g", g=NGRP),
                axis=AX.X, op=ALU.add)
            nc.scalar.activation(lnS, srow, AF.Ln)
            # w*cos_ii = pa * inv_p * (inv_a[0][:, 0:8]/KA) * w
            nc.vector.tensor_mul(cosd, pa, inv_p)
            nc.vector.tensor_mul(cosd, cosd, inv_a[0][:, 0:NT_P])
            nc.vector.tensor_scalar_mul(cosd, cosd, float(w) / KA)
            nc.vector.scalar_tensor_tensor(
                out=rowloss, in0=cosd, scalar=-1.0, in1=lnS,
                op0=ALU.mult, op1=ALU.add)
            nc.vector.reduce_sum(rsum, rowloss, axis=AX.X)
            with tc.tile_pool(name="psF", bufs=1, space="PSUM") as psF:
                pfin = psF.tile([1, 1], f32, tag="pfin")
                nc.tensor.matmul(pfin, rsum, ones, start=True, stop=True)
                nc.vector.tensor_copy(sc_out, pfin)
            nc.sync.dma_start(out=out_partial, in_=sc_out)

    nc.compile()
    return nc


def _get_nc(w: float, b: float):
    key = (float(w), float(b))
    if key not in _BUILD_CACHE:
        _BUILD_CACHE[key] = _build(float(w), float(b))
    return _BUILD_CACHE[key]


def build_in_maps(x):
    xa_full = np.ascontiguousarray(x[:, 1, :])
    in_maps = []
    for c in range(NCORES):
        r0 = c * RPC
        # rotate so each core starts streaming at its own shard: spreads
        # the 8 cores' concurrent HBM reads across the whole xa region
        # (they otherwise contend on identical addresses), and makes the
        # core's own anchor rows land in its group-0 tiles (no xad input).
        xa_rot = np.roll(xa_full, -r0, axis=0)
        in_maps.append({
            "xp": np.ascontiguousarray(x[r0:r0 + RPC, 0, :]),
            "xa": np.ascontiguousarray(xa_rot),
        })
    return in_maps


def kernel(x, w, b, epoch=None, **_unused):
    from concourse.bass_utils import run_bass_kernel_spmd

    x = np.asarray(x, dtype=np.float32)
    w_f = float(np.asarray(w))
    b_f = float(np.asarray(b))
    assert x.shape == (N, 2, D), x.shape

    nc = _get_nc(w_f, b_f)
    in_maps = build_in_maps(x)

    res = run_bass_kernel_spmd(nc, in_maps, list(range(NCORES)))
    total = 0.0
    for c in range(NCORES):
        total += float(res.results[c]["partial"][0, 0])
    loss = total / N
    return np.float32(loss)
